# revision 21
# baseline (speedup 1.0000x reference)
"""Trainium2 Bass kernel for nn_DinoGazeSpade (segment_reduce), 8 NeuronCores.

Distribution:
  - segment means + low-rank SPADE conv factor U: sharded by (layer,tap) unit
  - one-hot resize W of the segment map: sharded by (segment-range, image)
  - big gamma0 conv (3072 out-ch): sharded by output channels (384/core)
  - LayerNorm stats + 1x1-contraction partials merged into one AllReduce
Two collectives total: one AllGather (W + U), one AllReduce (partials).

Key algebra (derived from the reference model):
  painted/sem never materialize: sem[b,c,Y,X] = sum_s means[b,s,c]*W[b,s,Y,X],
  W = bilinear-antialias-resize of each segment's one-hot mask.
  h_l = relu(conv3x3(sem, ws_l) + bs) = relu(sum_tap U_tap^T @ W_tap + bs),
  U_tap[s,k] = sum_c means[s,c] ws_l[k,c,tap]  (contraction over 64 segs).
  beta0 (3072-ch conv) folds through the 1x1 conv w0: V = conv3x3(h0, wb0eff),
  wb0eff[o,k,tap] = sum_c w0[o,c] wb0[c,k,tap]; same fold gives T2 from wg0.
  z0pre = rho*(P1+T1) - rho*mu*(T2 + w0bsum) + V + K + bias0, with
    P1 = W0b @ x, T1[o,p] = sum_c w0[o,c] x[c,p] g[c,p],
    g = conv3x3(h0, wg0) (no bias), W0b = w0*(1+bg0), K = w0 @ bb0.
"""

from contextlib import ExitStack

import ml_dtypes
import numpy as np

import concourse.bass as bass
import concourse.tile as tile
from concourse import bacc, mybir
from concourse.bass_utils import run_bass_kernel_spmd

F32 = mybir.dt.float32
F32R = mybir.dt.float32r
BF = mybir.dt.bfloat16
I32 = mybir.dt.int32
AF = mybir.ActivationFunctionType
ALU = mybir.AluOpType
AX = mybir.AxisListType

B = 2
S = 64
HIMG = 336
HP = 24
NP = HP * HP
CD = 768
CM = 3072
RH = 48
NPIX = RH * RH
HID = 128
EPS = 1e-12
NC_N = 8
CSH = CM // NC_N      # 384
SSH = S // 4          # 16 segments per core
PW = RH + 2
NPAD = PW * PW
NT = 6                # pixel tiles: 8 rows of 48 = 384
NTW = 384

UNITS = [(l, t) for l in range(3) for t in range(9)]   # 27
UPC = 4

W_SEC = SSH * NPIX
U_SLOT = 2 * S * HID
AG_LEN = W_SEC + UPC * U_SLOT

AR_T1P1 = 0
AR_T2V = AR_T1P1 + 8 * 2 * NPIX
AR_STAT = AR_T2V + 16 * 2 * NPIX
AR_MISC = AR_STAT + 128 * 16
AR_LEN = AR_MISC + 8 * 4

USE_F32R = False


def resize_matrix(in_size: int, out_size: int) -> np.ndarray:
    """Port of jax.image.resize (bilinear, antialias=True) weight matrix.
    Returns (in_size, out_size)."""
    scale = out_size / in_size
    inv_scale = 1.0 / scale
    kernel_scale = max(inv_scale, 1.0)
    sample_f = (np.arange(out_size) + 0.5) * inv_scale - 0.5
    x = np.abs(sample_f[None, :] - np.arange(in_size)[:, None]) / kernel_scale
    weights = np.maximum(0.0, 1.0 - x)
    total = weights.sum(axis=0, keepdims=True)
    weights = np.where(
        np.abs(total) > 1000.0 * np.finfo(np.float32).eps,
        weights / np.where(total != 0, total, 1),
        0.0,
    )
    ok = (sample_f >= -0.5) & (sample_f <= in_size - 0.5)
    return np.where(ok[None, :], weights, 0.0).astype(np.float32)


def _r(ap):
    return ap.bitcast(F32R) if USE_F32R else ap


def build_kernel():
    nc = bacc.Bacc("TRN2", target_bir_lowering=False, debug=False,
                   num_devices=NC_N)

    def din(name, shape, dt=F32):
        return nc.declare_dram_parameter(name, list(shape), dt, isOutput=False)

    E = {}
    E["seg_my"] = din("seg_my", (HIMG, HIMG), I32)
    E["seg_p2"] = din("seg_p2", (2, NP), I32)
    E["fsemt"] = din("fsemt", (2, NP, CD))
    E["r_yt"] = din("r_yt", (HIMG, RH))
    E["r_ytb"] = din("r_ytb", (HIMG, RH), BF)
    E["sbase"] = din("sbase", (112, SSH))
    E["x_r"] = din("x_r", (CSH, 2 * NPIX), BF)
    E["wg0t"] = din("wg0t", (9, HID, CSH), BF)
    E["wg0_r"] = din("wg0_r", (CSH, HID * 9), BF)
    E["wb0_r"] = din("wb0_r", (CSH, HID * 9), BF)
    E["w0t_r"] = din("w0t_r", (CSH, 8), BF)
    E["bg0_r"] = din("bg0_r", (CSH, 1))
    E["bb0_r"] = din("bb0_r", (CSH, 1))
    E["wst_u"] = din("wst_u", (UPC, CD, HID))
    E["bs_all"] = din("bs_all", (3, HID, 1))
    E["gb1t"] = din("gb1t", (9, HID, 16), BF)
    E["gb2t"] = din("gb2t", (9, HID, 32), BF)
    E["gbias1"] = din("gbias1", (8, 2))
    E["gbias2"] = din("gbias2", (16, 2))
    E["w1t"] = din("w1t", (8, 16))
    E["w2t"] = din("w2t", (16, 1))
    E["b1_c"] = din("b1_c", (16, 1))
    E["b2_c"] = din("b2_c", (1, 1))
    E["bias0_c"] = din("bias0_c", (8, 1))
    E["out"] = nc.declare_dram_parameter("out", [B, 1, RH, RH], F32,
                                         isOutput=True)
    E["ag_in"] = nc.dram_tensor("ag_in", [AG_LEN], F32)
    E["ag_out"] = nc.dram_tensor("ag_out", [NC_N, AG_LEN], F32,
                                 addr_space="Shared")
    E["ar_in"] = nc.dram_tensor("ar_in", [AR_LEN], F32)
    E["ar_out"] = nc.dram_tensor("ar_out", [AR_LEN], F32, addr_space="Shared")
    E["eff_dram"] = nc.dram_tensor("eff_dram", [HID * 9, 16], BF)

    with tile.TileContext(nc, num_cores=NC_N) as tc:
        _body(nc, tc, E)
    nc.finalize()
    return nc


def _body(nc, tc, E):
    with ExitStack() as top:
        per = top.enter_context(tc.tile_pool(name="persist", bufs=1))

        # --- persistent small constants ---
        ident = per.tile([128, 128], F32)
        with tc.tile_pool(name="identp", bufs=1) as ip:
            ii0 = ip.tile([128, 128], I32)
            ii1 = ip.tile([128, 128], I32)
            nc.gpsimd.iota(ii0[:], pattern=[[1, 128]], base=0,
                           channel_multiplier=0)
            nc.gpsimd.iota(ii1[:], pattern=[[0, 128]], base=0,
                           channel_multiplier=1)
            nc.vector.tensor_tensor(ident[:], ii0[:], ii1[:], ALU.is_equal)
        ones_col = per.tile([128, 1], F32)
        nc.vector.memset(ones_col[:], 1.0)
        ones_row = per.tile([1, 128], F32)
        nc.vector.memset(ones_row[:], 1.0)
        ones_colb = per.tile([128, 1], BF)
        nc.vector.memset(ones_colb[:], 1.0)

        w_pad = [per.tile([S, NPAD], BF, name=f"wpad{i}") for i in range(B)]
        ag_out, ag_in, ar_in, ar_out = (E["ag_out"], E["ag_in"], E["ar_in"],
                                        E["ar_out"])

        # ================= PHASE 0: means, U units, W build =================
        with ExitStack() as p0:
            pool = p0.enter_context(tc.tile_pool(name="ph0", bufs=1))
            big = p0.enter_context(tc.tile_pool(name="ph0big", bufs=1))
            tp = p0.enter_context(tc.tile_pool(name="ph0t", bufs=3))
            ps = p0.enter_context(tc.tile_pool(name="ph0ps", bufs=2,
                                               space="PSUM"))

            iota_s = pool.tile([S, 1], F32)
            is_i = tp.tile([S, 1], I32, tag="isi")
            nc.gpsimd.iota(is_i[:], pattern=[[0, 1]], base=0,
                           channel_multiplier=1)
            nc.vector.tensor_copy(iota_s[:], is_i[:])

            # ---- segment means (both images), meansT (c, s) in 6 c-tiles ----
            meansT = [pool.tile([128, 6 * S], F32, name=f"meansT{i}") for i in range(B)]
            for b in range(B):
                segp_row = tp.tile([1, NP], I32, tag="segprow")
                nc.sync.dma_start(segp_row[:], E["seg_p2"][b : b + 1, :])
                segp_f = tp.tile([1, NP], F32, tag="segpf")
                nc.vector.tensor_copy(segp_f[:], segp_row[:])
                segb = tp.tile([S, NP], F32, tag="segb")
                for h in range(2):
                    sb_ps = ps.tile([S, NP // 2], F32, tag="p0a")
                    nc.tensor.matmul(sb_ps[:], ones_row[:, 0:S],
                                     segp_f[:, h * 288:(h + 1) * 288],
                                     start=True, stop=True)
                    nc.vector.tensor_copy(segb[:, h * 288:(h + 1) * 288],
                                          sb_ps[:])
                o2 = tp.tile([S, NP], F32, tag="o2")
                nc.vector.tensor_scalar(o2[:], segb[:], iota_s[:], None,
                                        ALU.is_equal)
                cnt = tp.tile([S, 1], F32, tag="cnt")
                nc.vector.tensor_reduce(cnt[:], o2[:], AX.X, ALU.add)
                nc.vector.tensor_scalar_max(cnt[:], cnt[:], 1.0)
                rec = tp.tile([S, 1], F32, tag="rec")
                nc.vector.reciprocal(rec[:], cnt[:])
                nc.vector.tensor_scalar_mul(o2[:], o2[:], rec[:])
                ot = pool.tile([128, 5 * S], F32, name=f"ot{b}", tag=f"ot{b}")
                for pc in range(5):
                    w = 128 if pc < 4 else 64
                    t_ps = ps.tile([128, S], F32, tag="p0a")
                    nc.tensor.transpose(t_ps[0:w, :],
                                        o2[:, pc * 128 : pc * 128 + w],
                                        ident[0:S, 0:S])
                    nc.vector.tensor_copy(ot[0:w, pc * S:(pc + 1) * S],
                                          t_ps[0:w, :])
                fst = big.tile([128, 5 * CD], F32, tag="fst")
                for pc in range(5):
                    w = 128 if pc < 4 else 64
                    nc.sync.dma_start(fst[0:w, pc * CD:(pc + 1) * CD],
                                      E["fsemt"][b, pc * 128 : pc * 128 + w, :])
                for ct in range(6):
                    m_ps = ps.tile([128, S], F32, tag="p0a")
                    for pc in range(5):
                        w = 128 if pc < 4 else 64
                        nc.tensor.matmul(
                            m_ps[:],
                            fst[0:w, pc * CD + ct * 128 : pc * CD + (ct + 1) * 128],
                            ot[0:w, pc * S:(pc + 1) * S],
                            start=(pc == 0), stop=(pc == 4))
                    nc.vector.tensor_copy(meansT[b][:, ct * S:(ct + 1) * S],
                                          m_ps[:])

            # ---- U units (4 slots/core) ----
            for u in range(UPC):
                wstt = tp.tile([128, 6 * HID], F32, tag="wstt")
                for ct in range(6):
                    nc.sync.dma_start(wstt[:, ct * HID:(ct + 1) * HID],
                                      E["wst_u"][u, ct * 128:(ct + 1) * 128, :])
                for b in range(B):
                    u_ps = ps.tile([S, HID], F32, tag="p0a")
                    for ct in range(6):
                        nc.tensor.matmul(u_ps[:],
                                         _r(meansT[b][:, ct * S:(ct + 1) * S]),
                                         _r(wstt[:, ct * HID:(ct + 1) * HID]),
                                         start=(ct == 0), stop=(ct == 5))
                    u_sb = tp.tile([S, HID], F32, tag="usb")
                    nc.vector.tensor_copy(u_sb[:], u_ps[:])
                    off = W_SEC + u * U_SLOT + b * S * HID
                    nc.sync.dma_start(
                        ag_in[off : off + S * HID].rearrange("(s k) -> s k",
                                                             s=S),
                        u_sb[:])

            # ---- W build: 16 segments of this core's image ----
            psw_ctx = tc.tile_pool(name="ph0psw", bufs=2, space="PSUM")
            psw = p0.enter_context(psw_ctx)
            segf = pool.tile([112, 3 * HIMG], F32)
            for yc in range(3):
                seg_i = tp.tile([112, HIMG], I32, tag="segi")
                nc.sync.dma_start(seg_i[:],
                                  E["seg_my"][yc * 112:(yc + 1) * 112, :])
                nc.vector.tensor_copy(segf[:, yc * HIMG:(yc + 1) * HIMG],
                                      seg_i[:])
            r_yt_sb = pool.tile([112, 3 * RH], F32)
            r_ytb_sb = pool.tile([112, 3 * RH], BF)
            for yc in range(3):
                nc.sync.dma_start(r_yt_sb[:, yc * RH:(yc + 1) * RH],
                                  E["r_yt"][yc * 112:(yc + 1) * 112, :])
                nc.sync.dma_start(r_ytb_sb[:, yc * RH:(yc + 1) * RH],
                                  E["r_ytb"][yc * 112:(yc + 1) * 112, :])
            sbase = pool.tile([112, SSH], F32)
            nc.sync.dma_start(sbase[:], E["sbase"][:, :])
            stag = pool.tile([RH, SSH * RH], F32)
            for si in range(SSH):
                a_ps = psw.tile([RH, HIMG], F32, tag="wa")
                for yc in range(3):
                    oh = tp.tile([112, HIMG], BF, tag="oh")
                    nc.vector.tensor_scalar(
                        oh[:], segf[:, yc * HIMG:(yc + 1) * HIMG],
                        sbase[:, si : si + 1], None, ALU.is_equal)
                    nc.tensor.matmul(a_ps[:],
                                     r_ytb_sb[:, yc * RH:(yc + 1) * RH],
                                     oh[:], start=(yc == 0),
                                     stop=(yc == 2))
                a_sb = tp.tile([RH, HIMG], F32, tag="asb")
                nc.scalar.activation(a_sb[:], a_ps[:], AF.Copy)
                w_ps = psw.tile([RH, RH], F32, tag="wps")
                for xc in range(3):
                    at_ps = psw.tile([112, RH], F32, tag="wa")
                    nc.tensor.transpose(at_ps[:],
                                        a_sb[:, xc * 112:(xc + 1) * 112],
                                        ident[0:RH, 0:RH])
                    at_sb = tp.tile([112, RH], F32, tag="atsb")
                    nc.vector.tensor_copy(at_sb[:], at_ps[:])
                    nc.tensor.matmul(w_ps[:], at_sb[:],
                                     r_yt_sb[:, xc * RH:(xc + 1) * RH],
                                     start=(xc == 0), stop=(xc == 2))
                nc.vector.tensor_copy(stag[:, si * RH:(si + 1) * RH], w_ps[:])
            nc.sync.dma_start(
                ag_in[0:W_SEC].rearrange("(s y x) -> y s x", s=SSH, y=RH,
                                         x=RH),
                stag[:])

        # ================= AllGather #1 =================
        nc.gpsimd.collective_compute(
            "AllGather", ALU.bypass, replica_groups=[list(range(NC_N))],
            ins=[ag_in[:]], outs=[ag_out[:]])

        # unpack W_pad (persistent, bf16 via f32 staging)
        with tc.tile_pool(name="wunp", bufs=1) as wup:
            for b in range(B):
                wpf = wup.tile([S, NPAD], F32, tag="wpf")
                nc.vector.memset(wpf[:], 0.0)
                for q in range(4):
                    core = 4 * b + q
                    nc.sync.dma_start(
                        wpf[q * SSH:(q + 1) * SSH, :].rearrange(
                            "s (y x) -> s y x", y=PW, x=PW)[
                            :, 1:1 + RH, 1:1 + RH],
                        ag_out[core, 0:W_SEC].rearrange("(s y x) -> s y x",
                                                        s=SSH, y=RH, x=RH))
                nc.vector.tensor_copy(w_pad[b][:], wpf[:])

        bs_sb = per.tile([HID, 3], F32)
        for l in range(3):
            nc.sync.dma_start(bs_sb[:, l : l + 1], E["bs_all"][l])

        def load_u(layer, pool_, tag):
            """load U for one layer, both images, cast bf16: [(64,9*128)]x2"""
            tiles = []
            for b in range(B):
                t = pool_.tile([S, 9 * HID], F32, name=f"{tag}{b}",
                               tag=f"{tag}{b}")
                for tap in range(9):
                    g = layer * 9 + tap
                    core, slot = g // UPC, g % UPC
                    off = W_SEC + slot * U_SLOT + b * S * HID
                    nc.sync.dma_start(
                        t[:, tap * HID:(tap + 1) * HID],
                        ag_out[core, off : off + S * HID].rearrange(
                            "(s k) -> s k", s=S))
                tb = pool_.tile([S, 9 * HID], BF, name=f"{tag}b{b}",
                                tag=f"{tag}b{b}")
                nc.vector.tensor_copy(tb[:], t[:])
                tiles.append(tb)
            return tiles

        def hconv(layer, dst_pad, u_tiles, psp):
            """write relu(conv3x3(sem, ws_l) + bs_l) into padded dst."""
            for b in range(B):
                base = b * NPAD
                nc.vector.memset(dst_pad[:, base : base + PW], 0.0)
                nc.vector.memset(dst_pad[:, base + NPAD - PW : base + NPAD],
                                 0.0)
                pv = dst_pad[:, base : base + NPAD].rearrange(
                    "k (y x) -> k y x", y=PW, x=PW)
                nc.vector.memset(pv[:, 1:PW - 1, 0:1], 0.0)
                nc.vector.memset(pv[:, 1:PW - 1, PW - 1:PW], 0.0)
            for b in range(B):
                wv = w_pad[b][:].rearrange("s (y x) -> s y x", y=PW, x=PW)
                for nt in range(NT):
                    h_ps = psp.tile([HID, NTW], F32, tag="hps")
                    for tap in range(9):
                        dy, dx = tap // 3, tap % 3
                        rhs = wv[:, dy + nt * 8 : dy + nt * 8 + 8,
                                 dx : dx + RH]
                        nc.tensor.matmul(
                            h_ps[:],
                            _r(u_tiles[b][:, tap * HID:(tap + 1) * HID]),
                            _r(rhs), start=(tap == 0), stop=(tap == 8))
                    dst = dst_pad[:, b * NPAD:(b + 1) * NPAD].rearrange(
                        "k (y x) -> k y x", y=PW, x=PW)[
                        :, 1 + nt * 8 : 1 + nt * 8 + 8, 1 : 1 + RH]
                    nc.scalar.activation(dst, h_ps[:], AF.Relu,
                                         bias=bs_sb[:, layer : layer + 1])

        # ================= PHASE 1: h0, folds, gamma0 partials =================
        with ExitStack() as p1:
            pool = p1.enter_context(tc.tile_pool(name="ph1", bufs=1))
            tp = p1.enter_context(tc.tile_pool(name="ph1t", bufs=3))

            # ---- W0T / W0bT / misc ----
            w0t_sb = pool.tile([128, 3 * 8], BF)
            w0bt_sb = pool.tile([128, 3 * 8], BF)
            bg0_sb = tp.tile([128, 3], F32, tag="bg0")
            bb0_sb = pool.tile([128, 3], F32)
            bb0_b = pool.tile([128, 3], BF)
            for ct in range(3):
                nc.sync.dma_start(w0t_sb[:, ct * 8:(ct + 1) * 8],
                                  E["w0t_r"][ct * 128:(ct + 1) * 128, :])
                nc.sync.dma_start(bg0_sb[:, ct : ct + 1],
                                  E["bg0_r"][ct * 128:(ct + 1) * 128])
                nc.sync.dma_start(bb0_sb[:, ct : ct + 1],
                                  E["bb0_r"][ct * 128:(ct + 1) * 128])
            onep = tp.tile([128, 3], F32, tag="onep")
            nc.vector.tensor_scalar_add(onep[:], bg0_sb[:], 1.0)
            nc.vector.tensor_copy(bb0_b[:], bb0_sb[:])
            for ct in range(3):
                nc.vector.tensor_scalar_mul(w0bt_sb[:, ct * 8:(ct + 1) * 8],
                                            w0t_sb[:, ct * 8:(ct + 1) * 8],
                                            onep[:, ct : ct + 1])
            psm_cm = tc.tile_pool(name="ph1psm", bufs=1, space="PSUM")
            psm = p1.enter_context(psm_cm)
            ms1 = psm.tile([8, 1], F32, tag="ms1")
            ms2 = psm.tile([8, 1], F32, tag="ms2")
            for ct in range(3):
                nc.tensor.matmul(ms1[:], w0bt_sb[:, ct * 8:(ct + 1) * 8],
                                 ones_colb[:, :], start=(ct == 0),
                                 stop=(ct == 2))
                nc.tensor.matmul(ms2[:], w0t_sb[:, ct * 8:(ct + 1) * 8],
                                 bb0_b[:, ct : ct + 1], start=(ct == 0),
                                 stop=(ct == 2))
            misc_sb = tp.tile([8, 4], F32, tag="miscsb")
            nc.vector.memset(misc_sb[:], 0.0)
            nc.vector.tensor_copy(misc_sb[:, 0:1], ms1[:])
            nc.vector.tensor_copy(misc_sb[:, 1:2], ms2[:])
            nc.sync.dma_start(
                ar_in[AR_MISC : AR_MISC + 32].rearrange("(o c) -> o c", o=8),
                misc_sb[:])

            # ---- eff = [wg0eff | wb0eff] transposed, via DRAM roundtrip ----
            for mt in range(9):
                e_ps = psm.tile([128, 16], F32, tag="eps")
                for ct in range(3):
                    wgrow = tp.tile([128, 128], BF, tag="wgrow")
                    nc.sync.dma_start(wgrow[:],
                                      E["wg0_r"][ct * 128:(ct + 1) * 128,
                                                 mt * 128:(mt + 1) * 128])
                    wbrow = tp.tile([128, 128], BF, tag="wbrow")
                    nc.sync.dma_start(wbrow[:],
                                      E["wb0_r"][ct * 128:(ct + 1) * 128,
                                                 mt * 128:(mt + 1) * 128])
                    nc.tensor.matmul(e_ps[:, 0:8], _r(wgrow[:]),
                                     _r(w0t_sb[:, ct * 8:(ct + 1) * 8]),
                                     start=(ct == 0), stop=(ct == 2))
                    nc.tensor.matmul(e_ps[:, 8:16], _r(wbrow[:]),
                                     _r(w0t_sb[:, ct * 8:(ct + 1) * 8]),
                                     start=(ct == 0), stop=(ct == 2))
                e_sb = tp.tile([128, 16], BF, tag="esb")
                nc.vector.tensor_copy(e_sb[:], e_ps[:])
                nc.sync.dma_start(E["eff_dram"][mt * 128:(mt + 1) * 128, :],
                                  e_sb[:])
            efft = pool.tile([HID, 9 * 16], BF)
            nc.sync.dma_start(
                efft[:].rearrange("k (t c) -> k t c", t=9),
                E["eff_dram"].rearrange("(k t) c -> k t c", k=HID, t=9))

            # ---- stats + gamma0 partials + T1/P1 ----
            x_sb = [pool.tile([128, 2 * NPIX], BF, name=f"xsb{ct}",
                              tag=f"xsb{ct}") for ct in range(3)]
            wg_sb = [pool.tile([128, 9 * 128], BF, name=f"wgsb{ct}",
                              tag=f"wgsb{ct}") for ct in range(3)]
            for ct in range(3):
                nc.sync.dma_start(x_sb[ct][:],
                                  E["x_r"][ct * 128:(ct + 1) * 128, :])
                for tap in range(9):
                    nc.sync.dma_start(
                        wg_sb[ct][:, tap * 128:(tap + 1) * 128],
                        E["wg0t"][tap, :, ct * 128:(ct + 1) * 128])
            stat_sb = pool.tile([128, 16], F32)
            nc.vector.memset(stat_sb[:], 0.0)
            scratch = tp.tile([128, NPIX], F32, tag="scr")
            for ct in range(3):
                for b in range(B):
                    col = ct * 4 + 2 * b
                    nc.vector.tensor_reduce(
                        stat_sb[:, col : col + 1],
                        x_sb[ct][:, b * NPIX:(b + 1) * NPIX], AX.X, ALU.add)
                    nc.scalar.activation(
                        scratch[:], x_sb[ct][:, b * NPIX:(b + 1) * NPIX],
                        AF.Square, accum_out=stat_sb[:, col + 1 : col + 2])
            nc.sync.dma_start(
                ar_in[AR_STAT : AR_STAT + 128 * 16].rearrange("(p c) -> p c",
                                                              p=128),
                stat_sb[:])
            h0_pad = pool.tile([HID, 2 * NPAD], BF)
            u0 = load_u(0, pool, "u0")
            with tc.tile_pool(name="ph1psh", bufs=2, space="PSUM") as psh:
                hconv(0, h0_pad, u0, psh)

            # ---- T2|V conv from h0 ----
            h0v = [h0_pad[:, b * NPAD:(b + 1) * NPAD].rearrange(
                "k (y x) -> k y x", y=PW, x=PW) for b in range(B)]
            for b in range(B):
                for nt in range(NT):
                    tv_ps = psm.tile([16, NTW], F32, tag="tvps")
                    for tap in range(9):
                        dy, dx = tap // 3, tap % 3
                        rhs = h0v[b][:, dy + nt * 8 : dy + nt * 8 + 8,
                                     dx : dx + RH]
                        nc.tensor.matmul(tv_ps[:],
                                         _r(efft[:, tap * 16:(tap + 1) * 16]),
                                         _r(rhs), start=(tap == 0),
                                         stop=(tap == 8))
                    tv_sb = tp.tile([16, NTW], F32, tag="tvsb")
                    nc.scalar.activation(tv_sb[:], tv_ps[:], AF.Copy)
                    nc.sync.dma_start(
                        ar_in[AR_T2V : AR_T2V + 16 * 2 * NPIX].rearrange(
                            "(o p) -> o p", o=16)[
                            :, b * NPIX + nt * NTW : b * NPIX + (nt + 1) * NTW],
                        tv_sb[:])

            psg_cm = tc.tile_pool(name="ph1psg", bufs=2, space="PSUM")
            psg = p1.enter_context(psg_cm)
            for b in range(B):
                for nt in range(NT):
                    tp_ps = psg.tile([8, NTW], F32, tag="tpps")
                    for ct in range(3):
                        g_ps = psg.tile([128, NTW], F32, tag="gps")
                        for tap in range(9):
                            dy, dx = tap // 3, tap % 3
                            rhs = h0v[b][:, dy + nt * 8 : dy + nt * 8 + 8,
                                         dx : dx + RH]
                            nc.tensor.matmul(
                                g_ps[:],
                                _r(wg_sb[ct][:, tap * 128:(tap + 1) * 128]),
                                _r(rhs), start=(tap == 0), stop=(tap == 8))
                        g_sb = tp.tile([128, NTW], BF, tag="gsb")
                        nc.scalar.activation(g_sb[:], g_ps[:], AF.Copy)
                        xg = tp.tile([128, NTW], BF, tag="xg")
                        xsl = x_sb[ct][:, b * NPIX + nt * NTW :
                                       b * NPIX + (nt + 1) * NTW]
                        nc.vector.tensor_tensor(xg[:], xsl, g_sb[:], ALU.mult)
                        nc.tensor.matmul(tp_ps[:],
                                         _r(w0t_sb[:, ct * 8:(ct + 1) * 8]),
                                         _r(xg[:]), start=(ct == 0),
                                         stop=False)
                        nc.tensor.matmul(tp_ps[:],
                                         _r(w0bt_sb[:, ct * 8:(ct + 1) * 8]),
                                         _r(xsl), start=False,
                                         stop=(ct == 2))
                    tp_sb = tp.tile([8, NTW], F32, tag="tpsb")
                    nc.scalar.activation(tp_sb[:], tp_ps[:], AF.Copy)
                    nc.sync.dma_start(
                        ar_in[AR_T1P1 : AR_T1P1 + 8 * 2 * NPIX].rearrange(
                            "(o p) -> o p", o=8)[
                            :, b * NPIX + nt * NTW : b * NPIX + (nt + 1) * NTW],
                        tp_sb[:])

        # ================= AllReduce #2 =================
        nc.gpsimd.collective_compute(
            "AllReduce", ALU.add, replica_groups=[list(range(NC_N))],
            ins=[ar_in[:]], outs=[ar_out[:]])

        # ================= PHASE 2: finish (replicated) =================
        with ExitStack() as p2:
            pool = p2.enter_context(tc.tile_pool(name="ph2", bufs=1))
            big2 = p2.enter_context(tc.tile_pool(name="ph2big", bufs=1))
            tp = p2.enter_context(tc.tile_pool(name="ph2t", bufs=2))
            ps = p2.enter_context(tc.tile_pool(name="ph2ps", bufs=2,
                                               space="PSUM"))

            stat_f = tp.tile([128, 16], F32, tag="statf")
            misc_f = pool.tile([8, 4], F32)
            nc.sync.dma_start(stat_f[:],
                              ar_out[AR_STAT : AR_STAT + 128 * 16].rearrange(
                                  "(p c) -> p c", p=128))
            nc.sync.dma_start(misc_f[:],
                              ar_out[AR_MISC : AR_MISC + 32].rearrange(
                                  "(o c) -> o c", o=8))
            eps_sb = pool.tile([1, 1], F32)
            nc.vector.memset(eps_sb[:], float(EPS))
            tot_ps = ps.tile([1, 16], F32, tag="smallps")
            nc.tensor.matmul(tot_ps[:], ones_col[:, :], stat_f[:], start=True,
                             stop=True)
            tot = pool.tile([1, 16], F32)
            nc.vector.tensor_copy(tot[:], tot_ps[:])
            # combine over ct: s[k,b] = sum_ct tot[ct*4 + 2b + k]
            acc = pool.tile([1, 4], F32)
            nc.vector.tensor_tensor(acc[:], tot[:, 0:4], tot[:, 4:8], ALU.add)
            nc.vector.tensor_tensor(acc[:], acc[:], tot[:, 8:12], ALU.add)
            nelem = float(CM * NPIX)
            mu = pool.tile([1, B], F32)
            rho = pool.tile([1, B], F32)
            var = tp.tile([1, B], F32, tag="var")
            musq = tp.tile([1, B], F32, tag="musq")
            for b in range(B):
                nc.vector.tensor_scalar_mul(mu[:, b : b + 1],
                                            acc[:, 2 * b : 2 * b + 1],
                                            1.0 / nelem)
                nc.vector.tensor_scalar_mul(var[:, b : b + 1],
                                            acc[:, 2 * b + 1 : 2 * b + 2],
                                            1.0 / nelem)
            nc.vector.tensor_tensor(musq[:], mu[:], mu[:], ALU.mult)
            nc.vector.tensor_tensor(var[:], var[:], musq[:], ALU.subtract)
            sd = tp.tile([1, B], F32, tag="sd")
            nc.scalar.activation(sd[:], var[:], AF.Sqrt, bias=eps_sb[:])
            nc.vector.reciprocal(rho[:], sd[:])

            def bcast(src_ap, parts):
                bps = ps.tile([128, 1], F32, tag="smallps")
                nc.tensor.matmul(bps[0:parts, :], ones_row[:, 0:parts],
                                 src_ap, start=True, stop=True)
                sb = tp.tile([128, 1], F32, tag="bcsb")
                nc.vector.tensor_copy(sb[0:parts, :], bps[0:parts, :])
                return sb

            bias0_sb = pool.tile([8, 1], F32)
            nc.sync.dma_start(bias0_sb[:], E["bias0_c"][:])

            # ---- z0 ----
            z0 = pool.tile([8, 2 * NPIX], F32)
            t1p1v = ar_out[AR_T1P1 : AR_T1P1 + 8 * 2 * NPIX].rearrange(
                "(o p) -> o p", o=8)
            t2vv = ar_out[AR_T2V : AR_T2V + 16 * 2 * NPIX].rearrange(
                "(o p) -> o p", o=16)
            for b in range(B):
                t1p1 = big2.tile([8, NPIX], F32, tag="t1p1")
                t2_sb = big2.tile([8, NPIX], F32, tag="t2sb")
                v_sb = big2.tile([8, NPIX], F32, tag="vsb")
                nc.sync.dma_start(t1p1[:],
                                  t1p1v[:, b * NPIX:(b + 1) * NPIX])
                nc.sync.dma_start(t2_sb[:],
                                  t2vv[0:8, b * NPIX:(b + 1) * NPIX])
                nc.sync.dma_start(v_sb[:],
                                  t2vv[8:16, b * NPIX:(b + 1) * NPIX])
                rho_b = bcast(rho[:, b : b + 1], 8)
                rmu = tp.tile([1, 1], F32, tag="rmu")
                nc.vector.tensor_tensor(rmu[:], rho[:, b : b + 1],
                                        mu[:, b : b + 1], ALU.mult)
                nc.vector.tensor_scalar_mul(rmu[:], rmu[:], -1.0)
                nrmu_b = bcast(rmu[:], 8)
                cst = tp.tile([8, 1], F32, tag="cst")
                nc.vector.tensor_scalar(cst[:], misc_f[:, 0:1],
                                        nrmu_b[0:8, :], None, ALU.mult)
                nc.vector.tensor_tensor(cst[:], cst[:], misc_f[:, 1:2],
                                        ALU.add)
                nc.vector.tensor_tensor(cst[:], cst[:], bias0_sb[:], ALU.add)
                sl = slice(b * NPIX, (b + 1) * NPIX)
                tt = big2.tile([8, NPIX], F32, tag="zt1")
                nc.vector.tensor_scalar(tt[:], t1p1[:], rho_b[0:8, :], None,
                                        ALU.mult)
                t2s = big2.tile([8, NPIX], F32, tag="zt2")
                nc.vector.tensor_scalar(t2s[:], t2_sb[:], nrmu_b[0:8, :],
                                        None, ALU.mult)
                nc.vector.tensor_tensor(tt[:], tt[:], t2s[:], ALU.add)
                nc.vector.tensor_tensor(tt[:], tt[:], v_sb[:], ALU.add)
                nc.scalar.activation(tt[:], tt[:], AF.Exp, bias=cst[:])
                nc.scalar.activation(z0[:, sl], tt[:], AF.Ln, bias=1.0)

            # ---- small-layer helpers ----
            gb1b_sb = pool.tile([8, 2], F32)
            nc.sync.dma_start(gb1b_sb[:], E["gbias1"][:])
            gb2b_sb = pool.tile([16, 2], F32)
            nc.sync.dma_start(gb2b_sb[:], E["gbias2"][:])
            w1t_sb = pool.tile([8, 16], F32)
            nc.sync.dma_start(w1t_sb[:], E["w1t"][:])
            w2t_sb = pool.tile([16, 1], F32)
            nc.sync.dma_start(w2t_sb[:], E["w2t"][:])
            b1_sb = pool.tile([16, 1], F32)
            nc.sync.dma_start(b1_sb[:], E["b1_c"][:])
            b2_sb = pool.tile([1, 1], F32)
            nc.sync.dma_start(b2_sb[:], E["b2_c"][:])

            def layer_stats(z, ch):
                st = tp.tile([128, 4], F32, tag="lst")
                scr = big2.tile([16, NPIX], F32, tag="lscr")
                for b in range(B):
                    nc.vector.tensor_reduce(
                        st[0:ch, 2 * b : 2 * b + 1],
                        z[:, b * NPIX:(b + 1) * NPIX], AX.X, ALU.add)
                    nc.scalar.activation(
                        scr[0:ch, :], z[:, b * NPIX:(b + 1) * NPIX],
                        AF.Square,
                        accum_out=st[0:ch, 2 * b + 1 : 2 * b + 2])
                lt_ps = ps.tile([1, 4], F32, tag="smallps")
                nc.tensor.matmul(lt_ps[:], ones_col[0:ch, :], st[0:ch, :],
                                 start=True, stop=True)
                t4 = tp.tile([1, 4], F32, tag="lsttot")
                nc.vector.tensor_copy(t4[:], lt_ps[:])
                n = float(ch * NPIX)
                m_ = tp.tile([1, B], F32, tag="lmu")
                r_ = tp.tile([1, B], F32, tag="lrho")
                v_ = tp.tile([1, B], F32, tag="lvar")
                q_ = tp.tile([1, B], F32, tag="lmsq")
                for b in range(B):
                    nc.vector.tensor_scalar_mul(
                        m_[:, b : b + 1], t4[:, 2 * b : 2 * b + 1], 1.0 / n)
                    nc.vector.tensor_scalar_mul(
                        v_[:, b : b + 1], t4[:, 2 * b + 1 : 2 * b + 2],
                        1.0 / n)
                nc.vector.tensor_tensor(q_[:], m_[:], m_[:], ALU.mult)
                nc.vector.tensor_tensor(v_[:], v_[:], q_[:], ALU.subtract)
                s_ = tp.tile([1, B], F32, tag="lsd")
                nc.scalar.activation(s_[:], v_[:], AF.Sqrt, bias=eps_sb[:])
                nc.vector.reciprocal(r_[:], s_[:])
                return m_, r_

            def spade_small(z, ch_in, h_pad_, layer, gbt, gbias_sb, n_gb):
                mu_l, rho_l = layer_stats(z, ch_in)
                gbw = pool.tile([HID, 9 * n_gb], BF, name=f"gbw{layer}",
                                tag=f"gbw{layer}")
                nc.sync.dma_start(
                    gbw[:].rearrange("k (t c) -> k t c", t=9),
                    gbt[:, :, :].rearrange("t k c -> k t c"))
                y = pool.tile([16, 2 * NPIX], F32, tag="ybuf")
                hv = [h_pad_[:, b * NPAD:(b + 1) * NPAD].rearrange(
                    "k (y x) -> k y x", y=PW, x=PW) for b in range(B)]
                for b in range(B):
                    rho_b = bcast(rho_l[:, b : b + 1], ch_in)
                    nmr = tp.tile([1, 1], F32, tag="nmr")
                    nc.vector.tensor_tensor(nmr[:], rho_l[:, b : b + 1],
                                            mu_l[:, b : b + 1], ALU.mult)
                    nc.vector.tensor_scalar_mul(nmr[:], nmr[:], -1.0)
                    nmr_b = bcast(nmr[:], ch_in)
                    ln = big2.tile([16, NPIX], F32, tag="lnb")
                    nc.vector.tensor_scalar(
                        ln[0:ch_in, :], z[:, b * NPIX:(b + 1) * NPIX],
                        rho_b[0:ch_in, :], nmr_b[0:ch_in, :],
                        ALU.mult, ALU.add)
                    for nt in range(NT):
                        ga_ps = ps.tile([16, NTW], F32, tag="gaps", bufs=1)
                        be_ps = ps.tile([16, NTW], F32, tag="beps", bufs=1)
                        for tap in range(9):
                            dy, dx = tap // 3, tap % 3
                            rhs = hv[b][:, dy + nt * 8 : dy + nt * 8 + 8,
                                        dx : dx + RH]
                            nc.tensor.matmul(
                                ga_ps[0:ch_in, :],
                                _r(gbw[:, tap * n_gb : tap * n_gb + ch_in]),
                                _r(rhs), start=(tap == 0), stop=(tap == 8))
                            nc.tensor.matmul(
                                be_ps[0:ch_in, :],
                                _r(gbw[:, tap * n_gb + ch_in :
                                       (tap + 1) * n_gb]),
                                _r(rhs), start=(tap == 0), stop=(tap == 8))
                        ga_sb = tp.tile([16, NTW], F32, tag="gasb")
                        be_sb = tp.tile([16, NTW], F32, tag="besb")
                        nc.scalar.activation(
                            ga_sb[0:ch_in, :], ga_ps[0:ch_in, :], AF.Identity,
                            bias=gbias_sb[0:ch_in, 0:1])
                        nc.scalar.activation(
                            be_sb[0:ch_in, :], be_ps[0:ch_in, :], AF.Identity,
                            bias=gbias_sb[0:ch_in, 1:2])
                        ysl = y[0:ch_in, b * NPIX + nt * NTW :
                                b * NPIX + (nt + 1) * NTW]
                        lsl = ln[0:ch_in, nt * NTW : (nt + 1) * NTW]
                        nc.vector.tensor_tensor(ysl, lsl, ga_sb[0:ch_in, :],
                                                ALU.mult)
                        nc.vector.tensor_tensor(
                            ysl, ysl, be_sb[0:ch_in, :], ALU.add)
                return y

            h_pad = pool.tile([HID, 2 * NPAD], BF, tag="hpad12")
            u1 = load_u(1, pool, "u1")
            hconv(1, h_pad, u1, ps)
            y1 = spade_small(z0, 8, h_pad, 1, E["gb1t"], gb1b_sb, 16)
            z1 = pool.tile([16, 2 * NPIX], F32)
            for b in range(B):
                for nt in range(NT):
                    zp = ps.tile([16, NTW], F32, tag="zps")
                    nc.tensor.matmul(
                        zp[:], w1t_sb[:],
                        y1[0:8, b * NPIX + nt * NTW :
                           b * NPIX + (nt + 1) * NTW],
                        start=True, stop=True)
                    zex = tp.tile([16, NTW], F32, tag="zex")
                    nc.scalar.activation(zex[:], zp[:], AF.Exp, bias=b1_sb[:])
                    nc.scalar.activation(
                        z1[:, b * NPIX + nt * NTW :
                           b * NPIX + (nt + 1) * NTW],
                        zex[:], AF.Ln, bias=1.0)

            u2 = load_u(2, pool, "u2")
            hconv(2, h_pad, u2, ps)
            y2 = spade_small(z1, 16, h_pad, 2, E["gb2t"], gb2b_sb, 32)
            for b in range(B):
                for nt in range(NT):
                    zp = ps.tile([1, NTW], F32, tag="zps")
                    nc.tensor.matmul(
                        zp[:], w2t_sb[:],
                        y2[:, b * NPIX + nt * NTW :
                           b * NPIX + (nt + 1) * NTW],
                        start=True, stop=True)
                    ot = tp.tile([1, NTW], F32, tag="otile")
                    nc.scalar.activation(ot[:], zp[:], AF.Exp, bias=b2_sb[:])
                    nc.scalar.activation(ot[:], ot[:], AF.Ln, bias=1.0)
                    nc.sync.dma_start(
                        E["out"][b, 0].rearrange("y x -> (y x)")[
                            nt * NTW : (nt + 1) * NTW],
                        ot[:])


def _prep_inputs(inputs):
    R = resize_matrix(HIMG, RH)
    x = np.asarray(inputs["x_main"], np.float32)
    f_sem = np.asarray(inputs["f_sem"], np.float32)
    segmap = np.asarray(inputs["segmap"], np.int32)
    idx = np.arange(HP) * HIMG // HP
    seg_p2 = np.ascontiguousarray(
        segmap[:, idx][:, :, idx].reshape(2, NP)).astype(np.int32)
    fsemt = np.ascontiguousarray(f_sem.reshape(2, CD, NP).transpose(0, 2, 1))

    ws = [np.asarray(inputs[f"ws{l}"], np.float32) for l in range(3)]
    bs = np.stack([np.asarray(inputs[f"bs{l}"], np.float32)
                   for l in range(3)]).reshape(3, HID, 1)
    wst = [np.ascontiguousarray(w.reshape(HID, CD, 9).transpose(2, 1, 0))
           for w in ws]
    wg0 = np.asarray(inputs["wg0"], np.float32)
    wb0 = np.asarray(inputs["wb0"], np.float32)
    w0 = np.asarray(inputs["w0"], np.float32).reshape(8, CM)

    def pack_gb(wg, wb, nf):
        wgt = np.asarray(wg, np.float32).reshape(nf, HID, 9).transpose(2, 1, 0)
        wbt = np.asarray(wb, np.float32).reshape(nf, HID, 9).transpose(2, 1, 0)
        return np.ascontiguousarray(np.concatenate([wgt, wbt], axis=2))

    gb1t = pack_gb(inputs["wg1"], inputs["wb1"], 8)
    gb2t = pack_gb(inputs["wg2"], inputs["wb2"], 16)
    gbias1 = np.stack([1.0 + np.asarray(inputs["bg1"], np.float32),
                       np.asarray(inputs["bb1"], np.float32)], axis=1)
    gbias2 = np.stack([1.0 + np.asarray(inputs["bg2"], np.float32),
                       np.asarray(inputs["bb2"], np.float32)], axis=1)
    w1t = np.ascontiguousarray(
        np.asarray(inputs["w1"], np.float32).reshape(16, 8).T)
    w2t = np.ascontiguousarray(
        np.asarray(inputs["w2"], np.float32).reshape(1, 16).T)
    b1_c = np.asarray(inputs["bias1"], np.float32).reshape(16, 1)
    b2_c = np.asarray(inputs["bias2"], np.float32).reshape(1, 1)
    bias0_c = np.asarray(inputs["bias0"], np.float32).reshape(8, 1)

    maps = []
    for r in range(NC_N):
        c0 = r * CSH
        b_img = r // 4
        s0 = SSH * (r % 4)
        units = UNITS[r * UPC:(r + 1) * UPC] if r * UPC < 27 else []
        wst_units = np.zeros((UPC, CD, HID), np.float32)
        for i, (l, t) in enumerate(UNITS[r * UPC:min((r + 1) * UPC, 27)]):
            wst_units[i] = wst[l][t]
        m = {
            "seg_my": np.ascontiguousarray(segmap[b_img]),
            "seg_p2": seg_p2,
            "fsemt": fsemt,
            "r_yt": np.ascontiguousarray(R),
            "r_ytb": np.ascontiguousarray(R).astype(ml_dtypes.bfloat16),
            "sbase": np.ascontiguousarray(
                np.broadcast_to((s0 + np.arange(SSH, dtype=np.float32))[None,
                                :], (112, SSH))),
            "x_r": np.ascontiguousarray(
                x.reshape(2, CM, NPIX)[:, c0:c0 + CSH].transpose(1, 0, 2)
                .reshape(CSH, 2 * NPIX)).astype(ml_dtypes.bfloat16),
            "wg0t": np.ascontiguousarray(
                wg0[c0:c0 + CSH].reshape(CSH, HID, 9).transpose(2, 1, 0))
                .astype(ml_dtypes.bfloat16),
            "wg0_r": np.ascontiguousarray(
                wg0[c0:c0 + CSH].reshape(CSH, HID * 9)).astype(
                    ml_dtypes.bfloat16),
            "wb0_r": np.ascontiguousarray(
                wb0[c0:c0 + CSH].reshape(CSH, HID * 9)).astype(
                    ml_dtypes.bfloat16),
            "w0t_r": np.ascontiguousarray(w0[:, c0:c0 + CSH].T).astype(ml_dtypes.bfloat16),
            "bg0_r": np.asarray(inputs["bg0"],
                                np.float32)[c0:c0 + CSH].reshape(CSH, 1),
            "bb0_r": np.asarray(inputs["bb0"],
                                np.float32)[c0:c0 + CSH].reshape(CSH, 1),
            "wst_u": wst_units,
            "bs_all": np.ascontiguousarray(bs),
            "gb1t": gb1t.astype(ml_dtypes.bfloat16),
            "gb2t": gb2t.astype(ml_dtypes.bfloat16),
            "gbias1": np.ascontiguousarray(gbias1),
            "gbias2": np.ascontiguousarray(gbias2),
            "w1t": w1t, "w2t": w2t, "b1_c": b1_c, "b2_c": b2_c,
            "bias0_c": bias0_c,
        }
        maps.append(m)
    return maps


_NC_CACHE = {}


def kernel(**inputs):
    if "nc" not in _NC_CACHE:
        _NC_CACHE["nc"] = build_kernel()
    nc = _NC_CACHE["nc"]
    maps = _prep_inputs(inputs)
    res = run_bass_kernel_spmd(nc, maps, core_ids=list(range(NC_N)))
    return np.asarray(res.results[0]["out"])


# revision 22
# speedup vs baseline: 1.0290x; 1.0290x over previous
"""Trainium2 Bass kernel for nn_DinoGazeSpade (segment_reduce), 8 NeuronCores.

Distribution:
  - segment means + low-rank SPADE conv factor U: sharded by (layer,tap) unit
  - one-hot resize W of the segment map: sharded by (segment-range, image)
  - big gamma0 conv (3072 out-ch): sharded by output channels (384/core)
  - LayerNorm stats + 1x1-contraction partials merged into one AllReduce
Two collectives total: one AllGather (W + U), one AllReduce (partials).

Key algebra (derived from the reference model):
  painted/sem never materialize: sem[b,c,Y,X] = sum_s means[b,s,c]*W[b,s,Y,X],
  W = bilinear-antialias-resize of each segment's one-hot mask.
  h_l = relu(conv3x3(sem, ws_l) + bs) = relu(sum_tap U_tap^T @ W_tap + bs),
  U_tap[s,k] = sum_c means[s,c] ws_l[k,c,tap]  (contraction over 64 segs).
  beta0 (3072-ch conv) folds through the 1x1 conv w0: V = conv3x3(h0, wb0eff),
  wb0eff[o,k,tap] = sum_c w0[o,c] wb0[c,k,tap]; same fold gives T2 from wg0.
  z0pre = rho*(P1+T1) - rho*mu*(T2 + w0bsum) + V + K + bias0, with
    P1 = W0b @ x, T1[o,p] = sum_c w0[o,c] x[c,p] g[c,p],
    g = conv3x3(h0, wg0) (no bias), W0b = w0*(1+bg0), K = w0 @ bb0.
"""

from contextlib import ExitStack

import ml_dtypes
import numpy as np

import concourse.bass as bass
import concourse.tile as tile
from concourse import bacc, mybir
from concourse.bass_utils import run_bass_kernel_spmd

F32 = mybir.dt.float32
F32R = mybir.dt.float32r
BF = mybir.dt.bfloat16
I32 = mybir.dt.int32
AF = mybir.ActivationFunctionType
ALU = mybir.AluOpType
AX = mybir.AxisListType

B = 2
S = 64
HIMG = 336
HP = 24
NP = HP * HP
CD = 768
CM = 3072
RH = 48
NPIX = RH * RH
HID = 128
EPS = 1e-12
NC_N = 8
CSH = CM // NC_N      # 384
SSH = S // 4          # 16 segments per core
PW = RH + 2
NPAD = PW * PW
NT = 6                # pixel tiles: 8 rows of 48 = 384
NTW = 384

UNITS = [(l, t) for l in range(3) for t in range(9)]   # 27
UPC = 4

W_SEC = SSH * NPIX
U_SLOT = 2 * S * HID
AG_LEN = W_SEC + UPC * U_SLOT

AR_T1P1 = 0
AR_T2V = AR_T1P1 + 8 * 2 * NPIX
AR_STAT = AR_T2V + 16 * 2 * NPIX
AR_MISC = AR_STAT + 128 * 16
AR_LEN = AR_MISC + 8 * 4

USE_F32R = False


def resize_matrix(in_size: int, out_size: int) -> np.ndarray:
    """Port of jax.image.resize (bilinear, antialias=True) weight matrix.
    Returns (in_size, out_size)."""
    scale = out_size / in_size
    inv_scale = 1.0 / scale
    kernel_scale = max(inv_scale, 1.0)
    sample_f = (np.arange(out_size) + 0.5) * inv_scale - 0.5
    x = np.abs(sample_f[None, :] - np.arange(in_size)[:, None]) / kernel_scale
    weights = np.maximum(0.0, 1.0 - x)
    total = weights.sum(axis=0, keepdims=True)
    weights = np.where(
        np.abs(total) > 1000.0 * np.finfo(np.float32).eps,
        weights / np.where(total != 0, total, 1),
        0.0,
    )
    ok = (sample_f >= -0.5) & (sample_f <= in_size - 0.5)
    return np.where(ok[None, :], weights, 0.0).astype(np.float32)


def _r(ap):
    return ap.bitcast(F32R) if USE_F32R else ap


def build_kernel():
    nc = bacc.Bacc("TRN2", target_bir_lowering=False, debug=False,
                   num_devices=NC_N)

    def din(name, shape, dt=F32):
        return nc.declare_dram_parameter(name, list(shape), dt, isOutput=False)

    E = {}
    E["seg_my"] = din("seg_my", (HIMG, HIMG), I32)
    E["seg_p2"] = din("seg_p2", (2, NP), I32)
    E["fsemt"] = din("fsemt", (2, NP, CD))
    E["r_yt"] = din("r_yt", (HIMG, RH))
    E["r_ytb"] = din("r_ytb", (HIMG, RH), BF)
    E["sbase"] = din("sbase", (112, SSH))
    E["x_r"] = din("x_r", (CSH, 2 * NPIX), BF)
    E["wg0t"] = din("wg0t", (9, HID, CSH), BF)
    E["wg0_r"] = din("wg0_r", (CSH, HID * 9), BF)
    E["wb0_r"] = din("wb0_r", (CSH, HID * 9), BF)
    E["w0t_r"] = din("w0t_r", (CSH, 8), BF)
    E["bg0_r"] = din("bg0_r", (CSH, 1))
    E["bb0_r"] = din("bb0_r", (CSH, 1))
    E["wst_u"] = din("wst_u", (UPC, CD, HID))
    E["bs_all"] = din("bs_all", (3, HID, 1))
    E["gb1t"] = din("gb1t", (9, HID, 16), BF)
    E["gb2t"] = din("gb2t", (9, HID, 32), BF)
    E["gbias1"] = din("gbias1", (8, 2))
    E["gbias2"] = din("gbias2", (16, 2))
    E["w1t"] = din("w1t", (8, 16))
    E["w2t"] = din("w2t", (16, 1))
    E["b1_c"] = din("b1_c", (16, 1))
    E["b2_c"] = din("b2_c", (1, 1))
    E["bias0_c"] = din("bias0_c", (8, 1))
    E["out"] = nc.declare_dram_parameter("out", [B, 1, RH, RH], F32,
                                         isOutput=True)
    E["ag_in"] = nc.dram_tensor("ag_in", [AG_LEN], F32)
    E["ag_out"] = nc.dram_tensor("ag_out", [NC_N, AG_LEN], F32,
                                 addr_space="Shared")
    E["ar_in"] = nc.dram_tensor("ar_in", [AR_LEN], F32)
    E["ar_out"] = nc.dram_tensor("ar_out", [AR_LEN], F32, addr_space="Shared")
    E["eff_dram"] = nc.dram_tensor("eff_dram", [HID * 9, 16], BF)

    with tile.TileContext(nc, num_cores=NC_N) as tc:
        _body(nc, tc, E)
    nc.finalize()
    return nc


def _body(nc, tc, E):
    with ExitStack() as top:
        per = top.enter_context(tc.tile_pool(name="persist", bufs=1))

        # --- persistent small constants ---
        ident = per.tile([128, 128], F32)
        with tc.tile_pool(name="identp", bufs=1) as ip:
            ii0 = ip.tile([128, 128], I32)
            ii1 = ip.tile([128, 128], I32)
            nc.gpsimd.iota(ii0[:], pattern=[[1, 128]], base=0,
                           channel_multiplier=0)
            nc.gpsimd.iota(ii1[:], pattern=[[0, 128]], base=0,
                           channel_multiplier=1)
            nc.vector.tensor_tensor(ident[:], ii0[:], ii1[:], ALU.is_equal)
        ones_col = per.tile([128, 1], F32)
        nc.vector.memset(ones_col[:], 1.0)
        ones_row = per.tile([1, 128], F32)
        nc.vector.memset(ones_row[:], 1.0)
        ones_colb = per.tile([128, 1], BF)
        nc.vector.memset(ones_colb[:], 1.0)

        w_pad = [per.tile([S, NPAD], BF, name=f"wpad{i}") for i in range(B)]
        ag_out, ag_in, ar_in, ar_out = (E["ag_out"], E["ag_in"], E["ar_in"],
                                        E["ar_out"])

        # ================= PHASE 0: means, U units, W build =================
        with ExitStack() as p0:
            pool = p0.enter_context(tc.tile_pool(name="ph0", bufs=1))
            big = p0.enter_context(tc.tile_pool(name="ph0big", bufs=1))
            tp = p0.enter_context(tc.tile_pool(name="ph0t", bufs=3))
            ps = p0.enter_context(tc.tile_pool(name="ph0ps", bufs=2,
                                               space="PSUM"))

            iota_s = pool.tile([S, 1], F32)
            is_i = tp.tile([S, 1], I32, tag="isi")
            nc.gpsimd.iota(is_i[:], pattern=[[0, 1]], base=0,
                           channel_multiplier=1)
            nc.vector.tensor_copy(iota_s[:], is_i[:])

            # ---- segment means (both images), meansT (c, s) in 6 c-tiles ----
            meansT = [pool.tile([128, 6 * S], F32, name=f"meansT{i}") for i in range(B)]
            for b in range(B):
                segp_row = tp.tile([1, NP], I32, tag="segprow")
                nc.sync.dma_start(segp_row[:], E["seg_p2"][b : b + 1, :])
                segp_f = tp.tile([1, NP], F32, tag="segpf")
                nc.vector.tensor_copy(segp_f[:], segp_row[:])
                segb = tp.tile([S, NP], F32, tag="segb")
                for h in range(2):
                    sb_ps = ps.tile([S, NP // 2], F32, tag="p0a")
                    nc.tensor.matmul(sb_ps[:], ones_row[:, 0:S],
                                     segp_f[:, h * 288:(h + 1) * 288],
                                     start=True, stop=True)
                    nc.vector.tensor_copy(segb[:, h * 288:(h + 1) * 288],
                                          sb_ps[:])
                o2 = tp.tile([S, NP], F32, tag="o2")
                nc.vector.tensor_scalar(o2[:], segb[:], iota_s[:], None,
                                        ALU.is_equal)
                cnt = tp.tile([S, 1], F32, tag="cnt")
                nc.vector.tensor_reduce(cnt[:], o2[:], AX.X, ALU.add)
                nc.vector.tensor_scalar_max(cnt[:], cnt[:], 1.0)
                rec = tp.tile([S, 1], F32, tag="rec")
                nc.vector.reciprocal(rec[:], cnt[:])
                nc.vector.tensor_scalar_mul(o2[:], o2[:], rec[:])
                ot = pool.tile([128, 5 * S], F32, name=f"ot{b}", tag=f"ot{b}")
                for pc in range(5):
                    w = 128 if pc < 4 else 64
                    t_ps = ps.tile([128, S], F32, tag="p0a")
                    nc.tensor.transpose(t_ps[0:w, :],
                                        o2[:, pc * 128 : pc * 128 + w],
                                        ident[0:S, 0:S])
                    nc.vector.tensor_copy(ot[0:w, pc * S:(pc + 1) * S],
                                          t_ps[0:w, :])
                fst = big.tile([128, 5 * CD], F32, tag="fst")
                for pc in range(5):
                    w = 128 if pc < 4 else 64
                    nc.sync.dma_start(fst[0:w, pc * CD:(pc + 1) * CD],
                                      E["fsemt"][b, pc * 128 : pc * 128 + w, :])
                for ct in range(6):
                    m_ps = ps.tile([128, S], F32, tag="p0a")
                    for pc in range(5):
                        w = 128 if pc < 4 else 64
                        nc.tensor.matmul(
                            m_ps[:],
                            fst[0:w, pc * CD + ct * 128 : pc * CD + (ct + 1) * 128],
                            ot[0:w, pc * S:(pc + 1) * S],
                            start=(pc == 0), stop=(pc == 4))
                    nc.vector.tensor_copy(meansT[b][:, ct * S:(ct + 1) * S],
                                          m_ps[:])

            # ---- U units (4 slots/core) ----
            for u in range(UPC):
                wstt = tp.tile([128, 6 * HID], F32, tag="wstt")
                for ct in range(6):
                    nc.sync.dma_start(wstt[:, ct * HID:(ct + 1) * HID],
                                      E["wst_u"][u, ct * 128:(ct + 1) * 128, :])
                for b in range(B):
                    u_ps = ps.tile([S, HID], F32, tag="p0a")
                    for ct in range(6):
                        nc.tensor.matmul(u_ps[:],
                                         _r(meansT[b][:, ct * S:(ct + 1) * S]),
                                         _r(wstt[:, ct * HID:(ct + 1) * HID]),
                                         start=(ct == 0), stop=(ct == 5))
                    u_sb = tp.tile([S, HID], F32, tag="usb")
                    nc.vector.tensor_copy(u_sb[:], u_ps[:])
                    off = W_SEC + u * U_SLOT + b * S * HID
                    nc.sync.dma_start(
                        ag_in[off : off + S * HID].rearrange("(s k) -> s k",
                                                             s=S),
                        u_sb[:])

            # ---- W build: 16 segments of this core's image ----
            psw_ctx = tc.tile_pool(name="ph0psw", bufs=2, space="PSUM")
            psw = p0.enter_context(psw_ctx)
            segf = pool.tile([112, 3 * HIMG], F32)
            for yc in range(3):
                seg_i = tp.tile([112, HIMG], I32, tag="segi")
                nc.sync.dma_start(seg_i[:],
                                  E["seg_my"][yc * 112:(yc + 1) * 112, :])
                nc.vector.tensor_copy(segf[:, yc * HIMG:(yc + 1) * HIMG],
                                      seg_i[:])
            r_yt_sb = pool.tile([112, 3 * RH], F32)
            r_ytb_sb = pool.tile([112, 3 * RH], BF)
            for yc in range(3):
                nc.sync.dma_start(r_yt_sb[:, yc * RH:(yc + 1) * RH],
                                  E["r_yt"][yc * 112:(yc + 1) * 112, :])
                nc.sync.dma_start(r_ytb_sb[:, yc * RH:(yc + 1) * RH],
                                  E["r_ytb"][yc * 112:(yc + 1) * 112, :])
            sbase = pool.tile([112, SSH], F32)
            nc.sync.dma_start(sbase[:], E["sbase"][:, :])
            stag = pool.tile([RH, SSH * RH], F32)
            for si in range(SSH):
                a_ps = psw.tile([RH, HIMG], F32, tag="wa")
                for yc in range(3):
                    oh = tp.tile([112, HIMG], BF, tag="oh")
                    nc.vector.tensor_scalar(
                        oh[:], segf[:, yc * HIMG:(yc + 1) * HIMG],
                        sbase[:, si : si + 1], None, ALU.is_equal)
                    nc.tensor.matmul(a_ps[:],
                                     r_ytb_sb[:, yc * RH:(yc + 1) * RH],
                                     oh[:], start=(yc == 0),
                                     stop=(yc == 2))
                a_sb = tp.tile([RH, HIMG], F32, tag="asb")
                nc.scalar.activation(a_sb[:], a_ps[:], AF.Copy)
                w_ps = psw.tile([RH, RH], F32, tag="wps")
                for xc in range(3):
                    at_ps = psw.tile([112, RH], F32, tag="wa")
                    nc.tensor.transpose(at_ps[:],
                                        a_sb[:, xc * 112:(xc + 1) * 112],
                                        ident[0:RH, 0:RH])
                    at_sb = tp.tile([112, RH], F32, tag="atsb")
                    nc.vector.tensor_copy(at_sb[:], at_ps[:])
                    nc.tensor.matmul(w_ps[:], at_sb[:],
                                     r_yt_sb[:, xc * RH:(xc + 1) * RH],
                                     start=(xc == 0), stop=(xc == 2))
                nc.vector.tensor_copy(stag[:, si * RH:(si + 1) * RH], w_ps[:])
            nc.sync.dma_start(
                ag_in[0:W_SEC].rearrange("(s y x) -> y s x", s=SSH, y=RH,
                                         x=RH),
                stag[:])

        # ================= AllGather #1 =================
        nc.gpsimd.collective_compute(
            "AllGather", ALU.bypass, replica_groups=[list(range(NC_N))],
            ins=[ag_in[:]], outs=[ag_out[:]])

        # unpack W_pad (persistent, bf16 via f32 staging)
        with tc.tile_pool(name="wunp", bufs=1) as wup:
            for b in range(B):
                wpf = wup.tile([S, NPAD], F32, tag="wpf")
                nc.vector.memset(wpf[:], 0.0)
                for q in range(4):
                    core = 4 * b + q
                    nc.sync.dma_start(
                        wpf[q * SSH:(q + 1) * SSH, :].rearrange(
                            "s (y x) -> s y x", y=PW, x=PW)[
                            :, 1:1 + RH, 1:1 + RH],
                        ag_out[core, 0:W_SEC].rearrange("(s y x) -> s y x",
                                                        s=SSH, y=RH, x=RH))
                nc.vector.tensor_copy(w_pad[b][:], wpf[:])

        bs_sb = per.tile([HID, 3], F32)
        for l in range(3):
            nc.sync.dma_start(bs_sb[:, l : l + 1], E["bs_all"][l])

        def load_u(layer, pool_, tag):
            """load U for one layer, both images, cast bf16: [(64,9*128)]x2"""
            tiles = []
            for b in range(B):
                t = pool_.tile([S, 9 * HID], F32, name=f"{tag}{b}",
                               tag=f"{tag}{b}")
                for tap in range(9):
                    g = layer * 9 + tap
                    core, slot = g // UPC, g % UPC
                    off = W_SEC + slot * U_SLOT + b * S * HID
                    nc.sync.dma_start(
                        t[:, tap * HID:(tap + 1) * HID],
                        ag_out[core, off : off + S * HID].rearrange(
                            "(s k) -> s k", s=S))
                tb = pool_.tile([S, 9 * HID], BF, name=f"{tag}b{b}",
                                tag=f"{tag}b{b}")
                nc.vector.tensor_copy(tb[:], t[:])
                tiles.append(tb)
            return tiles

        def hconv(layer, dst_pad, u_tiles, psp):
            """write relu(conv3x3(sem, ws_l) + bs_l) into padded dst."""
            for b in range(B):
                base = b * NPAD
                nc.vector.memset(dst_pad[:, base : base + PW], 0.0)
                nc.vector.memset(dst_pad[:, base + NPAD - PW : base + NPAD],
                                 0.0)
                pv = dst_pad[:, base : base + NPAD].rearrange(
                    "k (y x) -> k y x", y=PW, x=PW)
                nc.vector.memset(pv[:, 1:PW - 1, 0:1], 0.0)
                nc.vector.memset(pv[:, 1:PW - 1, PW - 1:PW], 0.0)
            for b in range(B):
                wv = w_pad[b][:].rearrange("s (y x) -> s y x", y=PW, x=PW)
                for nt in range(NT):
                    h_ps = psp.tile([HID, NTW], F32, tag="hps")
                    for tap in range(9):
                        dy, dx = tap // 3, tap % 3
                        rhs = wv[:, dy + nt * 8 : dy + nt * 8 + 8,
                                 dx : dx + RH]
                        nc.tensor.matmul(
                            h_ps[:],
                            _r(u_tiles[b][:, tap * HID:(tap + 1) * HID]),
                            _r(rhs), start=(tap == 0), stop=(tap == 8))
                    dst = dst_pad[:, b * NPAD:(b + 1) * NPAD].rearrange(
                        "k (y x) -> k y x", y=PW, x=PW)[
                        :, 1 + nt * 8 : 1 + nt * 8 + 8, 1 : 1 + RH]
                    nc.scalar.activation(dst, h_ps[:], AF.Relu,
                                         bias=bs_sb[:, layer : layer + 1])

        # ================= PHASE 1: h0, folds, gamma0 partials =================
        with ExitStack() as p1:
            pool = p1.enter_context(tc.tile_pool(name="ph1", bufs=1))
            tp = p1.enter_context(tc.tile_pool(name="ph1t", bufs=3))

            h0_pad = pool.tile([HID, 2 * NPAD], BF)
            u0 = load_u(0, pool, "u0")
            with tc.tile_pool(name="ph1psh", bufs=2, space="PSUM") as psh:
                hconv(0, h0_pad, u0, psh)

            # ---- W0T / W0bT / misc ----
            w0t_sb = pool.tile([128, 3 * 8], BF)
            w0bt_sb = pool.tile([128, 3 * 8], BF)
            bg0_sb = tp.tile([128, 3], F32, tag="bg0")
            bb0_sb = pool.tile([128, 3], F32)
            bb0_b = pool.tile([128, 3], BF)
            for ct in range(3):
                nc.sync.dma_start(w0t_sb[:, ct * 8:(ct + 1) * 8],
                                  E["w0t_r"][ct * 128:(ct + 1) * 128, :])
                nc.sync.dma_start(bg0_sb[:, ct : ct + 1],
                                  E["bg0_r"][ct * 128:(ct + 1) * 128])
                nc.sync.dma_start(bb0_sb[:, ct : ct + 1],
                                  E["bb0_r"][ct * 128:(ct + 1) * 128])
            onep = tp.tile([128, 3], F32, tag="onep")
            nc.vector.tensor_scalar_add(onep[:], bg0_sb[:], 1.0)
            nc.vector.tensor_copy(bb0_b[:], bb0_sb[:])
            for ct in range(3):
                nc.vector.tensor_scalar_mul(w0bt_sb[:, ct * 8:(ct + 1) * 8],
                                            w0t_sb[:, ct * 8:(ct + 1) * 8],
                                            onep[:, ct : ct + 1])
            psm_cm = tc.tile_pool(name="ph1psm", bufs=1, space="PSUM")
            psm = p1.enter_context(psm_cm)
            ms1 = psm.tile([8, 1], F32, tag="ms1")
            ms2 = psm.tile([8, 1], F32, tag="ms2")
            for ct in range(3):
                nc.tensor.matmul(ms1[:], w0bt_sb[:, ct * 8:(ct + 1) * 8],
                                 ones_colb[:, :], start=(ct == 0),
                                 stop=(ct == 2))
                nc.tensor.matmul(ms2[:], w0t_sb[:, ct * 8:(ct + 1) * 8],
                                 bb0_b[:, ct : ct + 1], start=(ct == 0),
                                 stop=(ct == 2))
            misc_sb = tp.tile([8, 4], F32, tag="miscsb")
            nc.vector.memset(misc_sb[:], 0.0)
            nc.vector.tensor_copy(misc_sb[:, 0:1], ms1[:])
            nc.vector.tensor_copy(misc_sb[:, 1:2], ms2[:])
            nc.sync.dma_start(
                ar_in[AR_MISC : AR_MISC + 32].rearrange("(o c) -> o c", o=8),
                misc_sb[:])

            # ---- eff = [wg0eff | wb0eff] transposed, via DRAM roundtrip ----
            for mt in range(9):
                e_ps = psm.tile([128, 16], F32, tag="eps")
                for ct in range(3):
                    wgrow = tp.tile([128, 128], BF, tag="wgrow")
                    nc.sync.dma_start(wgrow[:],
                                      E["wg0_r"][ct * 128:(ct + 1) * 128,
                                                 mt * 128:(mt + 1) * 128])
                    wbrow = tp.tile([128, 128], BF, tag="wbrow")
                    nc.sync.dma_start(wbrow[:],
                                      E["wb0_r"][ct * 128:(ct + 1) * 128,
                                                 mt * 128:(mt + 1) * 128])
                    nc.tensor.matmul(e_ps[:, 0:8], _r(wgrow[:]),
                                     _r(w0t_sb[:, ct * 8:(ct + 1) * 8]),
                                     start=(ct == 0), stop=(ct == 2))
                    nc.tensor.matmul(e_ps[:, 8:16], _r(wbrow[:]),
                                     _r(w0t_sb[:, ct * 8:(ct + 1) * 8]),
                                     start=(ct == 0), stop=(ct == 2))
                e_sb = tp.tile([128, 16], BF, tag="esb")
                nc.vector.tensor_copy(e_sb[:], e_ps[:])
                nc.sync.dma_start(E["eff_dram"][mt * 128:(mt + 1) * 128, :],
                                  e_sb[:])
            efft = pool.tile([HID, 9 * 16], BF)
            nc.sync.dma_start(
                efft[:].rearrange("k (t c) -> k t c", t=9),
                E["eff_dram"].rearrange("(k t) c -> k t c", k=HID, t=9))

            # ---- T2|V conv from h0 ----
            h0v = [h0_pad[:, b * NPAD:(b + 1) * NPAD].rearrange(
                "k (y x) -> k y x", y=PW, x=PW) for b in range(B)]
            for b in range(B):
                for nt in range(NT):
                    tv_ps = psm.tile([16, NTW], F32, tag="tvps")
                    for tap in range(9):
                        dy, dx = tap // 3, tap % 3
                        rhs = h0v[b][:, dy + nt * 8 : dy + nt * 8 + 8,
                                     dx : dx + RH]
                        nc.tensor.matmul(tv_ps[:],
                                         _r(efft[:, tap * 16:(tap + 1) * 16]),
                                         _r(rhs), start=(tap == 0),
                                         stop=(tap == 8))
                    tv_sb = tp.tile([16, NTW], F32, tag="tvsb")
                    nc.scalar.activation(tv_sb[:], tv_ps[:], AF.Copy)
                    nc.sync.dma_start(
                        ar_in[AR_T2V : AR_T2V + 16 * 2 * NPIX].rearrange(
                            "(o p) -> o p", o=16)[
                            :, b * NPIX + nt * NTW : b * NPIX + (nt + 1) * NTW],
                        tv_sb[:])

            # ---- stats + gamma0 partials + T1/P1 ----
            x_sb = [pool.tile([128, 2 * NPIX], BF, name=f"xsb{ct}",
                              tag=f"xsb{ct}") for ct in range(3)]
            wg_sb = [pool.tile([128, 9 * 128], BF, name=f"wgsb{ct}",
                              tag=f"wgsb{ct}") for ct in range(3)]
            for ct in range(3):
                nc.sync.dma_start(x_sb[ct][:],
                                  E["x_r"][ct * 128:(ct + 1) * 128, :])
                for tap in range(9):
                    nc.sync.dma_start(
                        wg_sb[ct][:, tap * 128:(tap + 1) * 128],
                        E["wg0t"][tap, :, ct * 128:(ct + 1) * 128])
            stat_sb = pool.tile([128, 16], F32)
            nc.vector.memset(stat_sb[:], 0.0)
            scratch = tp.tile([128, NPIX], F32, tag="scr")
            for ct in range(3):
                for b in range(B):
                    col = ct * 4 + 2 * b
                    nc.vector.tensor_reduce(
                        stat_sb[:, col : col + 1],
                        x_sb[ct][:, b * NPIX:(b + 1) * NPIX], AX.X, ALU.add)
                    nc.scalar.activation(
                        scratch[:], x_sb[ct][:, b * NPIX:(b + 1) * NPIX],
                        AF.Square, accum_out=stat_sb[:, col + 1 : col + 2])
            nc.sync.dma_start(
                ar_in[AR_STAT : AR_STAT + 128 * 16].rearrange("(p c) -> p c",
                                                              p=128),
                stat_sb[:])
            psg_cm = tc.tile_pool(name="ph1psg", bufs=2, space="PSUM")
            psg = p1.enter_context(psg_cm)
            for b in range(B):
                for nt in range(NT):
                    tp_ps = psg.tile([8, NTW], F32, tag="tpps")
                    for ct in range(3):
                        g_ps = psg.tile([128, NTW], F32, tag="gps")
                        for tap in range(9):
                            dy, dx = tap // 3, tap % 3
                            rhs = h0v[b][:, dy + nt * 8 : dy + nt * 8 + 8,
                                         dx : dx + RH]
                            nc.tensor.matmul(
                                g_ps[:],
                                _r(wg_sb[ct][:, tap * 128:(tap + 1) * 128]),
                                _r(rhs), start=(tap == 0), stop=(tap == 8))
                        g_sb = tp.tile([128, NTW], BF, tag="gsb")
                        nc.scalar.activation(g_sb[:], g_ps[:], AF.Copy)
                        xg = tp.tile([128, NTW], BF, tag="xg")
                        xsl = x_sb[ct][:, b * NPIX + nt * NTW :
                                       b * NPIX + (nt + 1) * NTW]
                        nc.vector.tensor_tensor(xg[:], xsl, g_sb[:], ALU.mult)
                        nc.tensor.matmul(tp_ps[:],
                                         _r(w0t_sb[:, ct * 8:(ct + 1) * 8]),
                                         _r(xg[:]), start=(ct == 0),
                                         stop=False)
                        nc.tensor.matmul(tp_ps[:],
                                         _r(w0bt_sb[:, ct * 8:(ct + 1) * 8]),
                                         _r(xsl), start=False,
                                         stop=(ct == 2))
                    tp_sb = tp.tile([8, NTW], F32, tag="tpsb")
                    nc.scalar.activation(tp_sb[:], tp_ps[:], AF.Copy)
                    nc.sync.dma_start(
                        ar_in[AR_T1P1 : AR_T1P1 + 8 * 2 * NPIX].rearrange(
                            "(o p) -> o p", o=8)[
                            :, b * NPIX + nt * NTW : b * NPIX + (nt + 1) * NTW],
                        tp_sb[:])

        # ================= AllReduce #2 =================
        nc.gpsimd.collective_compute(
            "AllReduce", ALU.add, replica_groups=[list(range(NC_N))],
            ins=[ar_in[:]], outs=[ar_out[:]])

        # ================= PHASE 2: finish (replicated) =================
        with ExitStack() as p2:
            pool = p2.enter_context(tc.tile_pool(name="ph2", bufs=1))
            big2 = p2.enter_context(tc.tile_pool(name="ph2big", bufs=1))
            tp = p2.enter_context(tc.tile_pool(name="ph2t", bufs=2))
            ps = p2.enter_context(tc.tile_pool(name="ph2ps", bufs=2,
                                               space="PSUM"))

            stat_f = tp.tile([128, 16], F32, tag="statf")
            misc_f = pool.tile([8, 4], F32)
            nc.sync.dma_start(stat_f[:],
                              ar_out[AR_STAT : AR_STAT + 128 * 16].rearrange(
                                  "(p c) -> p c", p=128))
            nc.sync.dma_start(misc_f[:],
                              ar_out[AR_MISC : AR_MISC + 32].rearrange(
                                  "(o c) -> o c", o=8))
            eps_sb = pool.tile([1, 1], F32)
            nc.vector.memset(eps_sb[:], float(EPS))
            tot_ps = ps.tile([1, 16], F32, tag="smallps")
            nc.tensor.matmul(tot_ps[:], ones_col[:, :], stat_f[:], start=True,
                             stop=True)
            tot = pool.tile([1, 16], F32)
            nc.vector.tensor_copy(tot[:], tot_ps[:])
            # combine over ct: s[k,b] = sum_ct tot[ct*4 + 2b + k]
            acc = pool.tile([1, 4], F32)
            nc.vector.tensor_tensor(acc[:], tot[:, 0:4], tot[:, 4:8], ALU.add)
            nc.vector.tensor_tensor(acc[:], acc[:], tot[:, 8:12], ALU.add)
            nelem = float(CM * NPIX)
            mu = pool.tile([1, B], F32)
            rho = pool.tile([1, B], F32)
            var = tp.tile([1, B], F32, tag="var")
            musq = tp.tile([1, B], F32, tag="musq")
            for b in range(B):
                nc.vector.tensor_scalar_mul(mu[:, b : b + 1],
                                            acc[:, 2 * b : 2 * b + 1],
                                            1.0 / nelem)
                nc.vector.tensor_scalar_mul(var[:, b : b + 1],
                                            acc[:, 2 * b + 1 : 2 * b + 2],
                                            1.0 / nelem)
            nc.vector.tensor_tensor(musq[:], mu[:], mu[:], ALU.mult)
            nc.vector.tensor_tensor(var[:], var[:], musq[:], ALU.subtract)
            sd = tp.tile([1, B], F32, tag="sd")
            nc.scalar.activation(sd[:], var[:], AF.Sqrt, bias=eps_sb[:])
            nc.vector.reciprocal(rho[:], sd[:])

            def bcast(src_ap, parts):
                bps = ps.tile([128, 1], F32, tag="smallps")
                nc.tensor.matmul(bps[0:parts, :], ones_row[:, 0:parts],
                                 src_ap, start=True, stop=True)
                sb = tp.tile([128, 1], F32, tag="bcsb")
                nc.vector.tensor_copy(sb[0:parts, :], bps[0:parts, :])
                return sb

            bias0_sb = pool.tile([8, 1], F32)
            nc.sync.dma_start(bias0_sb[:], E["bias0_c"][:])

            # ---- z0 ----
            z0 = pool.tile([8, 2 * NPIX], F32)
            t1p1v = ar_out[AR_T1P1 : AR_T1P1 + 8 * 2 * NPIX].rearrange(
                "(o p) -> o p", o=8)
            t2vv = ar_out[AR_T2V : AR_T2V + 16 * 2 * NPIX].rearrange(
                "(o p) -> o p", o=16)
            for b in range(B):
                t1p1 = big2.tile([8, NPIX], F32, tag="t1p1")
                t2_sb = big2.tile([8, NPIX], F32, tag="t2sb")
                v_sb = big2.tile([8, NPIX], F32, tag="vsb")
                nc.sync.dma_start(t1p1[:],
                                  t1p1v[:, b * NPIX:(b + 1) * NPIX])
                nc.sync.dma_start(t2_sb[:],
                                  t2vv[0:8, b * NPIX:(b + 1) * NPIX])
                nc.sync.dma_start(v_sb[:],
                                  t2vv[8:16, b * NPIX:(b + 1) * NPIX])
                rho_b = bcast(rho[:, b : b + 1], 8)
                rmu = tp.tile([1, 1], F32, tag="rmu")
                nc.vector.tensor_tensor(rmu[:], rho[:, b : b + 1],
                                        mu[:, b : b + 1], ALU.mult)
                nc.vector.tensor_scalar_mul(rmu[:], rmu[:], -1.0)
                nrmu_b = bcast(rmu[:], 8)
                cst = tp.tile([8, 1], F32, tag="cst")
                nc.vector.tensor_scalar(cst[:], misc_f[:, 0:1],
                                        nrmu_b[0:8, :], None, ALU.mult)
                nc.vector.tensor_tensor(cst[:], cst[:], misc_f[:, 1:2],
                                        ALU.add)
                nc.vector.tensor_tensor(cst[:], cst[:], bias0_sb[:], ALU.add)
                sl = slice(b * NPIX, (b + 1) * NPIX)
                tt = big2.tile([8, NPIX], F32, tag="zt1")
                nc.vector.tensor_scalar(tt[:], t1p1[:], rho_b[0:8, :], None,
                                        ALU.mult)
                t2s = big2.tile([8, NPIX], F32, tag="zt2")
                nc.vector.tensor_scalar(t2s[:], t2_sb[:], nrmu_b[0:8, :],
                                        None, ALU.mult)
                nc.vector.tensor_tensor(tt[:], tt[:], t2s[:], ALU.add)
                nc.vector.tensor_tensor(tt[:], tt[:], v_sb[:], ALU.add)
                nc.scalar.activation(tt[:], tt[:], AF.Exp, bias=cst[:])
                nc.scalar.activation(z0[:, sl], tt[:], AF.Ln, bias=1.0)

            # ---- small-layer helpers ----
            gb1b_sb = pool.tile([8, 2], F32)
            nc.sync.dma_start(gb1b_sb[:], E["gbias1"][:])
            gb2b_sb = pool.tile([16, 2], F32)
            nc.sync.dma_start(gb2b_sb[:], E["gbias2"][:])
            w1t_sb = pool.tile([8, 16], F32)
            nc.sync.dma_start(w1t_sb[:], E["w1t"][:])
            w2t_sb = pool.tile([16, 1], F32)
            nc.sync.dma_start(w2t_sb[:], E["w2t"][:])
            b1_sb = pool.tile([16, 1], F32)
            nc.sync.dma_start(b1_sb[:], E["b1_c"][:])
            b2_sb = pool.tile([1, 1], F32)
            nc.sync.dma_start(b2_sb[:], E["b2_c"][:])

            def layer_stats(z, ch):
                st = tp.tile([128, 4], F32, tag="lst")
                scr = big2.tile([16, NPIX], F32, tag="lscr")
                for b in range(B):
                    nc.vector.tensor_reduce(
                        st[0:ch, 2 * b : 2 * b + 1],
                        z[:, b * NPIX:(b + 1) * NPIX], AX.X, ALU.add)
                    nc.scalar.activation(
                        scr[0:ch, :], z[:, b * NPIX:(b + 1) * NPIX],
                        AF.Square,
                        accum_out=st[0:ch, 2 * b + 1 : 2 * b + 2])
                lt_ps = ps.tile([1, 4], F32, tag="smallps")
                nc.tensor.matmul(lt_ps[:], ones_col[0:ch, :], st[0:ch, :],
                                 start=True, stop=True)
                t4 = tp.tile([1, 4], F32, tag="lsttot")
                nc.vector.tensor_copy(t4[:], lt_ps[:])
                n = float(ch * NPIX)
                m_ = tp.tile([1, B], F32, tag="lmu")
                r_ = tp.tile([1, B], F32, tag="lrho")
                v_ = tp.tile([1, B], F32, tag="lvar")
                q_ = tp.tile([1, B], F32, tag="lmsq")
                for b in range(B):
                    nc.vector.tensor_scalar_mul(
                        m_[:, b : b + 1], t4[:, 2 * b : 2 * b + 1], 1.0 / n)
                    nc.vector.tensor_scalar_mul(
                        v_[:, b : b + 1], t4[:, 2 * b + 1 : 2 * b + 2],
                        1.0 / n)
                nc.vector.tensor_tensor(q_[:], m_[:], m_[:], ALU.mult)
                nc.vector.tensor_tensor(v_[:], v_[:], q_[:], ALU.subtract)
                s_ = tp.tile([1, B], F32, tag="lsd")
                nc.scalar.activation(s_[:], v_[:], AF.Sqrt, bias=eps_sb[:])
                nc.vector.reciprocal(r_[:], s_[:])
                return m_, r_

            def spade_small(z, ch_in, h_pad_, layer, gbt, gbias_sb, n_gb):
                mu_l, rho_l = layer_stats(z, ch_in)
                gbw = pool.tile([HID, 9 * n_gb], BF, name=f"gbw{layer}",
                                tag=f"gbw{layer}")
                nc.sync.dma_start(
                    gbw[:].rearrange("k (t c) -> k t c", t=9),
                    gbt[:, :, :].rearrange("t k c -> k t c"))
                y = pool.tile([16, 2 * NPIX], F32, tag="ybuf")
                hv = [h_pad_[:, b * NPAD:(b + 1) * NPAD].rearrange(
                    "k (y x) -> k y x", y=PW, x=PW) for b in range(B)]
                for b in range(B):
                    rho_b = bcast(rho_l[:, b : b + 1], ch_in)
                    nmr = tp.tile([1, 1], F32, tag="nmr")
                    nc.vector.tensor_tensor(nmr[:], rho_l[:, b : b + 1],
                                            mu_l[:, b : b + 1], ALU.mult)
                    nc.vector.tensor_scalar_mul(nmr[:], nmr[:], -1.0)
                    nmr_b = bcast(nmr[:], ch_in)
                    ln = big2.tile([16, NPIX], F32, tag="lnb")
                    nc.vector.tensor_scalar(
                        ln[0:ch_in, :], z[:, b * NPIX:(b + 1) * NPIX],
                        rho_b[0:ch_in, :], nmr_b[0:ch_in, :],
                        ALU.mult, ALU.add)
                    for nt in range(NT):
                        ga_ps = ps.tile([16, NTW], F32, tag="gaps", bufs=1)
                        be_ps = ps.tile([16, NTW], F32, tag="beps", bufs=1)
                        for tap in range(9):
                            dy, dx = tap // 3, tap % 3
                            rhs = hv[b][:, dy + nt * 8 : dy + nt * 8 + 8,
                                        dx : dx + RH]
                            nc.tensor.matmul(
                                ga_ps[0:ch_in, :],
                                _r(gbw[:, tap * n_gb : tap * n_gb + ch_in]),
                                _r(rhs), start=(tap == 0), stop=(tap == 8))
                            nc.tensor.matmul(
                                be_ps[0:ch_in, :],
                                _r(gbw[:, tap * n_gb + ch_in :
                                       (tap + 1) * n_gb]),
                                _r(rhs), start=(tap == 0), stop=(tap == 8))
                        ga_sb = tp.tile([16, NTW], F32, tag="gasb")
                        be_sb = tp.tile([16, NTW], F32, tag="besb")
                        nc.scalar.activation(
                            ga_sb[0:ch_in, :], ga_ps[0:ch_in, :], AF.Identity,
                            bias=gbias_sb[0:ch_in, 0:1])
                        nc.scalar.activation(
                            be_sb[0:ch_in, :], be_ps[0:ch_in, :], AF.Identity,
                            bias=gbias_sb[0:ch_in, 1:2])
                        ysl = y[0:ch_in, b * NPIX + nt * NTW :
                                b * NPIX + (nt + 1) * NTW]
                        lsl = ln[0:ch_in, nt * NTW : (nt + 1) * NTW]
                        nc.vector.tensor_tensor(ysl, lsl, ga_sb[0:ch_in, :],
                                                ALU.mult)
                        nc.vector.tensor_tensor(
                            ysl, ysl, be_sb[0:ch_in, :], ALU.add)
                return y

            h_pad = pool.tile([HID, 2 * NPAD], BF, tag="hpad12")
            u1 = load_u(1, pool, "u1")
            hconv(1, h_pad, u1, ps)
            y1 = spade_small(z0, 8, h_pad, 1, E["gb1t"], gb1b_sb, 16)
            z1 = pool.tile([16, 2 * NPIX], F32)
            for b in range(B):
                for nt in range(NT):
                    zp = ps.tile([16, NTW], F32, tag="zps")
                    nc.tensor.matmul(
                        zp[:], w1t_sb[:],
                        y1[0:8, b * NPIX + nt * NTW :
                           b * NPIX + (nt + 1) * NTW],
                        start=True, stop=True)
                    zex = tp.tile([16, NTW], F32, tag="zex")
                    nc.scalar.activation(zex[:], zp[:], AF.Exp, bias=b1_sb[:])
                    nc.scalar.activation(
                        z1[:, b * NPIX + nt * NTW :
                           b * NPIX + (nt + 1) * NTW],
                        zex[:], AF.Ln, bias=1.0)

            u2 = load_u(2, pool, "u2")
            hconv(2, h_pad, u2, ps)
            y2 = spade_small(z1, 16, h_pad, 2, E["gb2t"], gb2b_sb, 32)
            for b in range(B):
                for nt in range(NT):
                    zp = ps.tile([1, NTW], F32, tag="zps")
                    nc.tensor.matmul(
                        zp[:], w2t_sb[:],
                        y2[:, b * NPIX + nt * NTW :
                           b * NPIX + (nt + 1) * NTW],
                        start=True, stop=True)
                    ot = tp.tile([1, NTW], F32, tag="otile")
                    nc.scalar.activation(ot[:], zp[:], AF.Exp, bias=b2_sb[:])
                    nc.scalar.activation(ot[:], ot[:], AF.Ln, bias=1.0)
                    nc.sync.dma_start(
                        E["out"][b, 0].rearrange("y x -> (y x)")[
                            nt * NTW : (nt + 1) * NTW],
                        ot[:])


def _prep_inputs(inputs):
    R = resize_matrix(HIMG, RH)
    x = np.asarray(inputs["x_main"], np.float32)
    f_sem = np.asarray(inputs["f_sem"], np.float32)
    segmap = np.asarray(inputs["segmap"], np.int32)
    idx = np.arange(HP) * HIMG // HP
    seg_p2 = np.ascontiguousarray(
        segmap[:, idx][:, :, idx].reshape(2, NP)).astype(np.int32)
    fsemt = np.ascontiguousarray(f_sem.reshape(2, CD, NP).transpose(0, 2, 1))

    ws = [np.asarray(inputs[f"ws{l}"], np.float32) for l in range(3)]
    bs = np.stack([np.asarray(inputs[f"bs{l}"], np.float32)
                   for l in range(3)]).reshape(3, HID, 1)
    wst = [np.ascontiguousarray(w.reshape(HID, CD, 9).transpose(2, 1, 0))
           for w in ws]
    wg0 = np.asarray(inputs["wg0"], np.float32)
    wb0 = np.asarray(inputs["wb0"], np.float32)
    w0 = np.asarray(inputs["w0"], np.float32).reshape(8, CM)

    def pack_gb(wg, wb, nf):
        wgt = np.asarray(wg, np.float32).reshape(nf, HID, 9).transpose(2, 1, 0)
        wbt = np.asarray(wb, np.float32).reshape(nf, HID, 9).transpose(2, 1, 0)
        return np.ascontiguousarray(np.concatenate([wgt, wbt], axis=2))

    gb1t = pack_gb(inputs["wg1"], inputs["wb1"], 8)
    gb2t = pack_gb(inputs["wg2"], inputs["wb2"], 16)
    gbias1 = np.stack([1.0 + np.asarray(inputs["bg1"], np.float32),
                       np.asarray(inputs["bb1"], np.float32)], axis=1)
    gbias2 = np.stack([1.0 + np.asarray(inputs["bg2"], np.float32),
                       np.asarray(inputs["bb2"], np.float32)], axis=1)
    w1t = np.ascontiguousarray(
        np.asarray(inputs["w1"], np.float32).reshape(16, 8).T)
    w2t = np.ascontiguousarray(
        np.asarray(inputs["w2"], np.float32).reshape(1, 16).T)
    b1_c = np.asarray(inputs["bias1"], np.float32).reshape(16, 1)
    b2_c = np.asarray(inputs["bias2"], np.float32).reshape(1, 1)
    bias0_c = np.asarray(inputs["bias0"], np.float32).reshape(8, 1)

    maps = []
    for r in range(NC_N):
        c0 = r * CSH
        b_img = r // 4
        s0 = SSH * (r % 4)
        units = UNITS[r * UPC:(r + 1) * UPC] if r * UPC < 27 else []
        wst_units = np.zeros((UPC, CD, HID), np.float32)
        for i, (l, t) in enumerate(UNITS[r * UPC:min((r + 1) * UPC, 27)]):
            wst_units[i] = wst[l][t]
        m = {
            "seg_my": np.ascontiguousarray(segmap[b_img]),
            "seg_p2": seg_p2,
            "fsemt": fsemt,
            "r_yt": np.ascontiguousarray(R),
            "r_ytb": np.ascontiguousarray(R).astype(ml_dtypes.bfloat16),
            "sbase": np.ascontiguousarray(
                np.broadcast_to((s0 + np.arange(SSH, dtype=np.float32))[None,
                                :], (112, SSH))),
            "x_r": np.ascontiguousarray(
                x.reshape(2, CM, NPIX)[:, c0:c0 + CSH].transpose(1, 0, 2)
                .reshape(CSH, 2 * NPIX)).astype(ml_dtypes.bfloat16),
            "wg0t": np.ascontiguousarray(
                wg0[c0:c0 + CSH].reshape(CSH, HID, 9).transpose(2, 1, 0))
                .astype(ml_dtypes.bfloat16),
            "wg0_r": np.ascontiguousarray(
                wg0[c0:c0 + CSH].reshape(CSH, HID * 9)).astype(
                    ml_dtypes.bfloat16),
            "wb0_r": np.ascontiguousarray(
                wb0[c0:c0 + CSH].reshape(CSH, HID * 9)).astype(
                    ml_dtypes.bfloat16),
            "w0t_r": np.ascontiguousarray(w0[:, c0:c0 + CSH].T).astype(ml_dtypes.bfloat16),
            "bg0_r": np.asarray(inputs["bg0"],
                                np.float32)[c0:c0 + CSH].reshape(CSH, 1),
            "bb0_r": np.asarray(inputs["bb0"],
                                np.float32)[c0:c0 + CSH].reshape(CSH, 1),
            "wst_u": wst_units,
            "bs_all": np.ascontiguousarray(bs),
            "gb1t": gb1t.astype(ml_dtypes.bfloat16),
            "gb2t": gb2t.astype(ml_dtypes.bfloat16),
            "gbias1": np.ascontiguousarray(gbias1),
            "gbias2": np.ascontiguousarray(gbias2),
            "w1t": w1t, "w2t": w2t, "b1_c": b1_c, "b2_c": b2_c,
            "bias0_c": bias0_c,
        }
        maps.append(m)
    return maps


_NC_CACHE = {}


def kernel(**inputs):
    if "nc" not in _NC_CACHE:
        _NC_CACHE["nc"] = build_kernel()
    nc = _NC_CACHE["nc"]
    maps = _prep_inputs(inputs)
    res = run_bass_kernel_spmd(nc, maps, core_ids=list(range(NC_N)))
    return np.asarray(res.results[0]["out"])


# revision 24
# speedup vs baseline: 1.0541x; 1.0243x over previous
"""Trainium2 Bass kernel for nn_DinoGazeSpade (segment_reduce), 8 NeuronCores.

Distribution:
  - segment means + low-rank SPADE conv factor U: sharded by (layer,tap) unit
  - one-hot resize W of the segment map: sharded by (segment-range, image)
  - big gamma0 conv (3072 out-ch): sharded by output channels (384/core)
  - LayerNorm stats + 1x1-contraction partials merged into one AllReduce
Two collectives total: one AllGather (W + U), one AllReduce (partials).

Key algebra (derived from the reference model):
  painted/sem never materialize: sem[b,c,Y,X] = sum_s means[b,s,c]*W[b,s,Y,X],
  W = bilinear-antialias-resize of each segment's one-hot mask.
  h_l = relu(conv3x3(sem, ws_l) + bs) = relu(sum_tap U_tap^T @ W_tap + bs),
  U_tap[s,k] = sum_c means[s,c] ws_l[k,c,tap]  (contraction over 64 segs).
  beta0 (3072-ch conv) folds through the 1x1 conv w0: V = conv3x3(h0, wb0eff),
  wb0eff[o,k,tap] = sum_c w0[o,c] wb0[c,k,tap]; same fold gives T2 from wg0.
  z0pre = rho*(P1+T1) - rho*mu*(T2 + w0bsum) + V + K + bias0, with
    P1 = W0b @ x, T1[o,p] = sum_c w0[o,c] x[c,p] g[c,p],
    g = conv3x3(h0, wg0) (no bias), W0b = w0*(1+bg0), K = w0 @ bb0.
"""

from contextlib import ExitStack

import ml_dtypes
import numpy as np

import concourse.bass as bass
import concourse.tile as tile
from concourse import bacc, mybir
from concourse.bass_utils import run_bass_kernel_spmd

F32 = mybir.dt.float32
F32R = mybir.dt.float32r
BF = mybir.dt.bfloat16
I32 = mybir.dt.int32
AF = mybir.ActivationFunctionType
ALU = mybir.AluOpType
AX = mybir.AxisListType

B = 2
S = 64
HIMG = 336
HP = 24
NP = HP * HP
CD = 768
CM = 3072
RH = 48
NPIX = RH * RH
HID = 128
EPS = 1e-12
NC_N = 8
CSH = CM // NC_N      # 384
SSH = S // 4          # 16 segments per core
PW = RH + 2
NPAD = PW * PW
NT = 6                # pixel tiles: 8 rows of 48 = 384
NTW = 384

UNITS = [(l, t) for l in range(3) for t in range(9)]   # 27
UPC = 4

W_SEC = SSH * NPIX
U_SLOT = 2 * S * HID
AG_LEN = W_SEC + UPC * U_SLOT

AR_T1P1 = 0
AR_T2V = AR_T1P1 + 8 * 2 * NPIX
AR_STAT = AR_T2V + 16 * 2 * NPIX
AR_MISC = AR_STAT + 128 * 16
AR_LEN = AR_MISC + 8 * 4

USE_F32R = False


def resize_matrix(in_size: int, out_size: int) -> np.ndarray:
    """Port of jax.image.resize (bilinear, antialias=True) weight matrix.
    Returns (in_size, out_size)."""
    scale = out_size / in_size
    inv_scale = 1.0 / scale
    kernel_scale = max(inv_scale, 1.0)
    sample_f = (np.arange(out_size) + 0.5) * inv_scale - 0.5
    x = np.abs(sample_f[None, :] - np.arange(in_size)[:, None]) / kernel_scale
    weights = np.maximum(0.0, 1.0 - x)
    total = weights.sum(axis=0, keepdims=True)
    weights = np.where(
        np.abs(total) > 1000.0 * np.finfo(np.float32).eps,
        weights / np.where(total != 0, total, 1),
        0.0,
    )
    ok = (sample_f >= -0.5) & (sample_f <= in_size - 0.5)
    return np.where(ok[None, :], weights, 0.0).astype(np.float32)


def _r(ap):
    return ap.bitcast(F32R) if USE_F32R else ap


def build_kernel():
    nc = bacc.Bacc("TRN2", target_bir_lowering=False, debug=False,
                   num_devices=NC_N)

    def din(name, shape, dt=F32):
        return nc.declare_dram_parameter(name, list(shape), dt, isOutput=False)

    E = {}
    E["seg_my"] = din("seg_my", (HIMG, HIMG), I32)
    E["seg_p2"] = din("seg_p2", (2, NP), I32)
    E["fsemt"] = din("fsemt", (2, NP, CD))
    E["r_yt"] = din("r_yt", (HIMG, RH))
    E["r_ytb"] = din("r_ytb", (HIMG, RH), BF)
    E["sbase"] = din("sbase", (112, SSH))
    E["x_r"] = din("x_r", (CSH, 2 * NPIX), BF)
    E["wg0t"] = din("wg0t", (9, HID, CSH), BF)
    E["wg0_r"] = din("wg0_r", (CSH, HID * 9), BF)
    E["wb0_r"] = din("wb0_r", (CSH, HID * 9), BF)
    E["w0t_r"] = din("w0t_r", (CSH, 8), BF)
    E["bg0_r"] = din("bg0_r", (CSH, 1))
    E["bb0_r"] = din("bb0_r", (CSH, 1))
    E["wst_u"] = din("wst_u", (UPC, CD, HID))
    E["bs_all"] = din("bs_all", (3, HID, 1))
    E["gb1t"] = din("gb1t", (9, HID, 16), BF)
    E["gb2t"] = din("gb2t", (9, HID, 32), BF)
    E["gbias1"] = din("gbias1", (8, 2))
    E["gbias2"] = din("gbias2", (16, 2))
    E["w1t"] = din("w1t", (8, 16))
    E["w2t"] = din("w2t", (16, 1))
    E["b1_c"] = din("b1_c", (16, 1))
    E["b2_c"] = din("b2_c", (1, 1))
    E["bias0_c"] = din("bias0_c", (8, 1))
    E["out"] = nc.declare_dram_parameter("out", [B, 1, RH, RH], F32,
                                         isOutput=True)
    E["ag_in"] = nc.dram_tensor("ag_in", [AG_LEN], F32)
    E["ag_out"] = nc.dram_tensor("ag_out", [NC_N, AG_LEN], F32,
                                 addr_space="Shared")
    E["ar_in"] = nc.dram_tensor("ar_in", [AR_LEN], F32)
    E["ar_out"] = nc.dram_tensor("ar_out", [AR_LEN], F32, addr_space="Shared")
    E["eff_dram"] = nc.dram_tensor("eff_dram", [HID * 9, 16], BF)

    with tile.TileContext(nc, num_cores=NC_N) as tc:
        _body(nc, tc, E)
    nc.finalize()
    return nc


def _body(nc, tc, E):
    with ExitStack() as top:
        per = top.enter_context(tc.tile_pool(name="persist", bufs=1))

        # --- persistent small constants ---
        ident = per.tile([128, 128], F32)
        with tc.tile_pool(name="identp", bufs=1) as ip:
            ii0 = ip.tile([128, 128], I32)
            ii1 = ip.tile([128, 128], I32)
            nc.gpsimd.iota(ii0[:], pattern=[[1, 128]], base=0,
                           channel_multiplier=0)
            nc.gpsimd.iota(ii1[:], pattern=[[0, 128]], base=0,
                           channel_multiplier=1)
            nc.vector.tensor_tensor(ident[:], ii0[:], ii1[:], ALU.is_equal)
        ones_col = per.tile([128, 1], F32)
        nc.vector.memset(ones_col[:], 1.0)
        ones_row = per.tile([1, 128], F32)
        nc.vector.memset(ones_row[:], 1.0)
        ones_colb = per.tile([128, 1], BF)
        nc.vector.memset(ones_colb[:], 1.0)

        w_pad = [per.tile([S, NPAD], BF, name=f"wpad{i}") for i in range(B)]
        ag_out, ag_in, ar_in, ar_out = (E["ag_out"], E["ag_in"], E["ar_in"],
                                        E["ar_out"])

        # ================= PHASE 0: means, U units, W build =================
        with ExitStack() as p0:
            pool = p0.enter_context(tc.tile_pool(name="ph0", bufs=1))
            big = p0.enter_context(tc.tile_pool(name="ph0big", bufs=1))
            tp = p0.enter_context(tc.tile_pool(name="ph0t", bufs=3))
            ps = p0.enter_context(tc.tile_pool(name="ph0ps", bufs=2,
                                               space="PSUM"))

            iota_s = pool.tile([S, 1], F32)
            is_i = tp.tile([S, 1], I32, tag="isi")
            nc.gpsimd.iota(is_i[:], pattern=[[0, 1]], base=0,
                           channel_multiplier=1)
            nc.vector.tensor_copy(iota_s[:], is_i[:])

            # ---- segment means (both images), meansT (c, s) in 6 c-tiles ----
            meansT = [pool.tile([128, 6 * S], F32, name=f"meansT{i}") for i in range(B)]
            for b in range(B):
                segp_row = tp.tile([1, NP], I32, tag="segprow")
                nc.sync.dma_start(segp_row[:], E["seg_p2"][b : b + 1, :])
                segp_f = tp.tile([1, NP], F32, tag="segpf")
                nc.vector.tensor_copy(segp_f[:], segp_row[:])
                segb = tp.tile([S, NP], F32, tag="segb")
                for h in range(2):
                    sb_ps = ps.tile([S, NP // 2], F32, tag="p0a")
                    nc.tensor.matmul(sb_ps[:], ones_row[:, 0:S],
                                     segp_f[:, h * 288:(h + 1) * 288],
                                     start=True, stop=True)
                    nc.vector.tensor_copy(segb[:, h * 288:(h + 1) * 288],
                                          sb_ps[:])
                o2 = tp.tile([S, NP], F32, tag="o2")
                nc.vector.tensor_scalar(o2[:], segb[:], iota_s[:], None,
                                        ALU.is_equal)
                cnt = tp.tile([S, 1], F32, tag="cnt")
                nc.vector.tensor_reduce(cnt[:], o2[:], AX.X, ALU.add)
                nc.vector.tensor_scalar_max(cnt[:], cnt[:], 1.0)
                rec = tp.tile([S, 1], F32, tag="rec")
                nc.vector.reciprocal(rec[:], cnt[:])
                nc.vector.tensor_scalar_mul(o2[:], o2[:], rec[:])
                ot = pool.tile([128, 5 * S], F32, name=f"ot{b}", tag=f"ot{b}")
                for pc in range(5):
                    w = 128 if pc < 4 else 64
                    t_ps = ps.tile([128, S], F32, tag="p0a")
                    nc.tensor.transpose(t_ps[0:w, :],
                                        o2[:, pc * 128 : pc * 128 + w],
                                        ident[0:S, 0:S])
                    nc.vector.tensor_copy(ot[0:w, pc * S:(pc + 1) * S],
                                          t_ps[0:w, :])
                fst = big.tile([128, 5 * CD], F32, tag="fst")
                for pc in range(5):
                    w = 128 if pc < 4 else 64
                    nc.sync.dma_start(fst[0:w, pc * CD:(pc + 1) * CD],
                                      E["fsemt"][b, pc * 128 : pc * 128 + w, :])
                for ct in range(6):
                    m_ps = ps.tile([128, S], F32, tag="p0a")
                    for pc in range(5):
                        w = 128 if pc < 4 else 64
                        nc.tensor.matmul(
                            m_ps[:],
                            fst[0:w, pc * CD + ct * 128 : pc * CD + (ct + 1) * 128],
                            ot[0:w, pc * S:(pc + 1) * S],
                            start=(pc == 0), stop=(pc == 4))
                    nc.vector.tensor_copy(meansT[b][:, ct * S:(ct + 1) * S],
                                          m_ps[:])

            # ---- U units (4 slots/core) ----
            for u in range(UPC):
                wstt = tp.tile([128, 6 * HID], F32, tag="wstt")
                for ct in range(6):
                    nc.sync.dma_start(wstt[:, ct * HID:(ct + 1) * HID],
                                      E["wst_u"][u, ct * 128:(ct + 1) * 128, :])
                for b in range(B):
                    u_ps = ps.tile([S, HID], F32, tag="p0a")
                    for ct in range(6):
                        nc.tensor.matmul(u_ps[:],
                                         _r(meansT[b][:, ct * S:(ct + 1) * S]),
                                         _r(wstt[:, ct * HID:(ct + 1) * HID]),
                                         start=(ct == 0), stop=(ct == 5))
                    u_sb = tp.tile([S, HID], F32, tag="usb")
                    nc.vector.tensor_copy(u_sb[:], u_ps[:])
                    off = W_SEC + u * U_SLOT + b * S * HID
                    nc.sync.dma_start(
                        ag_in[off : off + S * HID].rearrange("(s k) -> s k",
                                                             s=S),
                        u_sb[:])

            # ---- W build: 16 segments of this core's image ----
            psw_ctx = tc.tile_pool(name="ph0psw", bufs=2, space="PSUM")
            psw = p0.enter_context(psw_ctx)
            segf = pool.tile([112, 3 * HIMG], F32)
            for yc in range(3):
                seg_i = tp.tile([112, HIMG], I32, tag="segi")
                nc.sync.dma_start(seg_i[:],
                                  E["seg_my"][yc * 112:(yc + 1) * 112, :])
                nc.vector.tensor_copy(segf[:, yc * HIMG:(yc + 1) * HIMG],
                                      seg_i[:])
            r_yt_sb = pool.tile([112, 3 * RH], F32)
            r_ytb_sb = pool.tile([112, 3 * RH], BF)
            for yc in range(3):
                nc.sync.dma_start(r_yt_sb[:, yc * RH:(yc + 1) * RH],
                                  E["r_yt"][yc * 112:(yc + 1) * 112, :])
                nc.sync.dma_start(r_ytb_sb[:, yc * RH:(yc + 1) * RH],
                                  E["r_ytb"][yc * 112:(yc + 1) * 112, :])
            sbase = pool.tile([112, SSH], F32)
            nc.sync.dma_start(sbase[:], E["sbase"][:, :])
            stag = pool.tile([RH, SSH * RH], F32)
            for si in range(SSH):
                a_ps = psw.tile([RH, HIMG], F32, tag="wa")
                for yc in range(3):
                    oh = tp.tile([112, HIMG], BF, tag="oh")
                    nc.vector.tensor_scalar(
                        oh[:], segf[:, yc * HIMG:(yc + 1) * HIMG],
                        sbase[:, si : si + 1], None, ALU.is_equal)
                    nc.tensor.matmul(a_ps[:],
                                     r_ytb_sb[:, yc * RH:(yc + 1) * RH],
                                     oh[:], start=(yc == 0),
                                     stop=(yc == 2))
                a_sb = tp.tile([RH, HIMG], F32, tag="asb")
                nc.scalar.activation(a_sb[:], a_ps[:], AF.Copy)
                w_ps = psw.tile([RH, RH], F32, tag="wps")
                for xc in range(3):
                    at_ps = psw.tile([112, RH], F32, tag="wa")
                    nc.tensor.transpose(at_ps[:],
                                        a_sb[:, xc * 112:(xc + 1) * 112],
                                        ident[0:RH, 0:RH])
                    at_sb = tp.tile([112, RH], F32, tag="atsb")
                    nc.vector.tensor_copy(at_sb[:], at_ps[:])
                    nc.tensor.matmul(w_ps[:], at_sb[:],
                                     r_yt_sb[:, xc * RH:(xc + 1) * RH],
                                     start=(xc == 0), stop=(xc == 2))
                nc.vector.tensor_copy(stag[:, si * RH:(si + 1) * RH], w_ps[:])
            nc.sync.dma_start(
                ag_in[0:W_SEC].rearrange("(s y x) -> y s x", s=SSH, y=RH,
                                         x=RH),
                stag[:])

        # ================= AllGather #1 =================
        nc.gpsimd.collective_compute(
            "AllGather", ALU.bypass, replica_groups=[list(range(NC_N))],
            ins=[ag_in[:]], outs=[ag_out[:]])

        # unpack W_pad (persistent, bf16 via f32 staging)
        with tc.tile_pool(name="wunp", bufs=1) as wup:
            for b in range(B):
                wpf = wup.tile([S, NPAD], F32, tag="wpf")
                nc.vector.memset(wpf[:], 0.0)
                for q in range(4):
                    core = 4 * b + q
                    nc.sync.dma_start(
                        wpf[q * SSH:(q + 1) * SSH, :].rearrange(
                            "s (y x) -> s y x", y=PW, x=PW)[
                            :, 1:1 + RH, 1:1 + RH],
                        ag_out[core, 0:W_SEC].rearrange("(s y x) -> s y x",
                                                        s=SSH, y=RH, x=RH))
                nc.vector.tensor_copy(w_pad[b][:], wpf[:])

        bs_sb = per.tile([HID, 3], F32)
        for l in range(3):
            nc.sync.dma_start(bs_sb[:, l : l + 1], E["bs_all"][l])

        def load_u(layer, pool_, tag):
            """load U for one layer, both images, cast bf16: [(64,9*128)]x2"""
            tiles = []
            for b in range(B):
                t = pool_.tile([S, 9 * HID], F32, name=f"{tag}{b}",
                               tag=f"{tag}{b}")
                for tap in range(9):
                    g = layer * 9 + tap
                    core, slot = g // UPC, g % UPC
                    off = W_SEC + slot * U_SLOT + b * S * HID
                    nc.sync.dma_start(
                        t[:, tap * HID:(tap + 1) * HID],
                        ag_out[core, off : off + S * HID].rearrange(
                            "(s k) -> s k", s=S))
                tb = pool_.tile([S, 9 * HID], BF, name=f"{tag}b{b}",
                                tag=f"{tag}b{b}")
                nc.vector.tensor_copy(tb[:], t[:])
                tiles.append(tb)
            return tiles

        def hconv(layer, dst_pad, u_tiles, psp):
            """write relu(conv3x3(sem, ws_l) + bs_l) into padded dst."""
            for b in range(B):
                base = b * NPAD
                nc.vector.memset(dst_pad[:, base : base + PW], 0.0)
                nc.vector.memset(dst_pad[:, base + NPAD - PW : base + NPAD],
                                 0.0)
                pv = dst_pad[:, base : base + NPAD].rearrange(
                    "k (y x) -> k y x", y=PW, x=PW)
                nc.vector.memset(pv[:, 1:PW - 1, 0:1], 0.0)
                nc.vector.memset(pv[:, 1:PW - 1, PW - 1:PW], 0.0)
            for b in range(B):
                wv = w_pad[b][:].rearrange("s (y x) -> s y x", y=PW, x=PW)
                for nt in range(NT):
                    h_ps = psp.tile([HID, NTW], F32, tag="hps")
                    for tap in range(9):
                        dy, dx = tap // 3, tap % 3
                        rhs = wv[:, dy + nt * 8 : dy + nt * 8 + 8,
                                 dx : dx + RH]
                        nc.tensor.matmul(
                            h_ps[:],
                            _r(u_tiles[b][:, tap * HID:(tap + 1) * HID]),
                            _r(rhs), start=(tap == 0), stop=(tap == 8))
                    dst = dst_pad[:, b * NPAD:(b + 1) * NPAD].rearrange(
                        "k (y x) -> k y x", y=PW, x=PW)[
                        :, 1 + nt * 8 : 1 + nt * 8 + 8, 1 : 1 + RH]
                    nc.scalar.activation(dst, h_ps[:], AF.Relu,
                                         bias=bs_sb[:, layer : layer + 1])

        # ================= PHASE 1: h0, folds, gamma0 partials =================
        with ExitStack() as p1:
            pool = p1.enter_context(tc.tile_pool(name="ph1", bufs=1))
            tp = p1.enter_context(tc.tile_pool(name="ph1t", bufs=3))

            h0_pad = pool.tile([HID, 2 * NPAD], BF)
            u0 = load_u(0, pool, "u0")
            with tc.tile_pool(name="ph1psh", bufs=2, space="PSUM") as psh:
                hconv(0, h0_pad, u0, psh)

            # ---- W0T / W0bT / misc ----
            w0t_sb = pool.tile([128, 3 * 8], BF)
            w0bt_sb = pool.tile([128, 3 * 8], BF)
            bg0_sb = tp.tile([128, 3], F32, tag="bg0")
            bb0_sb = pool.tile([128, 3], F32)
            bb0_b = pool.tile([128, 3], BF)
            for ct in range(3):
                nc.sync.dma_start(w0t_sb[:, ct * 8:(ct + 1) * 8],
                                  E["w0t_r"][ct * 128:(ct + 1) * 128, :])
                nc.sync.dma_start(bg0_sb[:, ct : ct + 1],
                                  E["bg0_r"][ct * 128:(ct + 1) * 128])
                nc.sync.dma_start(bb0_sb[:, ct : ct + 1],
                                  E["bb0_r"][ct * 128:(ct + 1) * 128])
            onep = tp.tile([128, 3], F32, tag="onep")
            nc.vector.tensor_scalar_add(onep[:], bg0_sb[:], 1.0)
            nc.vector.tensor_copy(bb0_b[:], bb0_sb[:])
            for ct in range(3):
                nc.vector.tensor_scalar_mul(w0bt_sb[:, ct * 8:(ct + 1) * 8],
                                            w0t_sb[:, ct * 8:(ct + 1) * 8],
                                            onep[:, ct : ct + 1])
            psm_cm = tc.tile_pool(name="ph1psm", bufs=1, space="PSUM")
            psm = p1.enter_context(psm_cm)
            ms1 = psm.tile([8, 1], F32, tag="ms1")
            ms2 = psm.tile([8, 1], F32, tag="ms2")
            for ct in range(3):
                nc.tensor.matmul(ms1[:], w0bt_sb[:, ct * 8:(ct + 1) * 8],
                                 ones_colb[:, :], start=(ct == 0),
                                 stop=(ct == 2))
                nc.tensor.matmul(ms2[:], w0t_sb[:, ct * 8:(ct + 1) * 8],
                                 bb0_b[:, ct : ct + 1], start=(ct == 0),
                                 stop=(ct == 2))
            misc_sb = tp.tile([8, 4], F32, tag="miscsb")
            nc.vector.memset(misc_sb[:], 0.0)
            nc.vector.tensor_copy(misc_sb[:, 0:1], ms1[:])
            nc.vector.tensor_copy(misc_sb[:, 1:2], ms2[:])
            nc.sync.dma_start(
                ar_in[AR_MISC : AR_MISC + 32].rearrange("(o c) -> o c", o=8),
                misc_sb[:])

            # ---- eff = [wg0eff | wb0eff] transposed, via DRAM roundtrip ----
            for mt in range(9):
                e_ps = psm.tile([128, 16], F32, tag="eps")
                for ct in range(3):
                    wgrow = tp.tile([128, 128], BF, tag="wgrow")
                    nc.sync.dma_start(wgrow[:],
                                      E["wg0_r"][ct * 128:(ct + 1) * 128,
                                                 mt * 128:(mt + 1) * 128])
                    wbrow = tp.tile([128, 128], BF, tag="wbrow")
                    nc.sync.dma_start(wbrow[:],
                                      E["wb0_r"][ct * 128:(ct + 1) * 128,
                                                 mt * 128:(mt + 1) * 128])
                    nc.tensor.matmul(e_ps[:, 0:8], _r(wgrow[:]),
                                     _r(w0t_sb[:, ct * 8:(ct + 1) * 8]),
                                     start=(ct == 0), stop=(ct == 2))
                    nc.tensor.matmul(e_ps[:, 8:16], _r(wbrow[:]),
                                     _r(w0t_sb[:, ct * 8:(ct + 1) * 8]),
                                     start=(ct == 0), stop=(ct == 2))
                e_sb = tp.tile([128, 16], BF, tag="esb")
                nc.vector.tensor_copy(e_sb[:], e_ps[:])
                nc.sync.dma_start(E["eff_dram"][mt * 128:(mt + 1) * 128, :],
                                  e_sb[:])
            efft = pool.tile([HID, 9 * 16], BF)
            nc.sync.dma_start(
                efft[:].rearrange("k (t c) -> k t c", t=9),
                E["eff_dram"].rearrange("(k t) c -> k t c", k=HID, t=9))

            # ---- T2|V conv from h0 ----
            h0v = [h0_pad[:, b * NPAD:(b + 1) * NPAD].rearrange(
                "k (y x) -> k y x", y=PW, x=PW) for b in range(B)]
            for b in range(B):
                for nt in range(NT):
                    tv_ps = psm.tile([16, NTW], F32, tag="tvps")
                    for tap in range(9):
                        dy, dx = tap // 3, tap % 3
                        rhs = h0v[b][:, dy + nt * 8 : dy + nt * 8 + 8,
                                     dx : dx + RH]
                        nc.tensor.matmul(tv_ps[:],
                                         _r(efft[:, tap * 16:(tap + 1) * 16]),
                                         _r(rhs), start=(tap == 0),
                                         stop=(tap == 8))
                    tv_sb = tp.tile([16, NTW], F32, tag="tvsb")
                    nc.scalar.activation(tv_sb[:], tv_ps[:], AF.Copy)
                    nc.sync.dma_start(
                        ar_in[AR_T2V : AR_T2V + 16 * 2 * NPIX].rearrange(
                            "(o p) -> o p", o=16)[
                            :, b * NPIX + nt * NTW : b * NPIX + (nt + 1) * NTW],
                        tv_sb[:])

            # ---- stats + gamma0 partials + T1/P1 ----
            x_sb = [pool.tile([128, 2 * NPIX], BF, name=f"xsb{ct}",
                              tag=f"xsb{ct}") for ct in range(3)]
            wg_sb = [pool.tile([128, 9 * 128], BF, name=f"wgsb{ct}",
                              tag=f"wgsb{ct}") for ct in range(3)]
            for ct in range(3):
                nc.sync.dma_start(x_sb[ct][:],
                                  E["x_r"][ct * 128:(ct + 1) * 128, :])
                for tap in range(9):
                    nc.sync.dma_start(
                        wg_sb[ct][:, tap * 128:(tap + 1) * 128],
                        E["wg0t"][tap, :, ct * 128:(ct + 1) * 128])
            stat_sb = pool.tile([128, 16], F32)
            nc.vector.memset(stat_sb[:], 0.0)
            scratch = tp.tile([128, NPIX], F32, tag="scr")
            for ct in range(3):
                for b in range(B):
                    col = ct * 4 + 2 * b
                    nc.vector.tensor_reduce(
                        stat_sb[:, col : col + 1],
                        x_sb[ct][:, b * NPIX:(b + 1) * NPIX], AX.X, ALU.add)
                    nc.scalar.activation(
                        scratch[:], x_sb[ct][:, b * NPIX:(b + 1) * NPIX],
                        AF.Square, accum_out=stat_sb[:, col + 1 : col + 2])
            nc.sync.dma_start(
                ar_in[AR_STAT : AR_STAT + 128 * 16].rearrange("(p c) -> p c",
                                                              p=128),
                stat_sb[:])
            psg_cm = tc.tile_pool(name="ph1psg", bufs=2, space="PSUM")
            psg = p1.enter_context(psg_cm)
            for b in range(B):
                for nt in range(NT):
                    tp_ps = psg.tile([8, NTW], F32, tag="tpps")
                    for ct in range(3):
                        g_ps = psg.tile([128, NTW], F32, tag="gps")
                        for tap in range(9):
                            dy, dx = tap // 3, tap % 3
                            rhs = h0v[b][:, dy + nt * 8 : dy + nt * 8 + 8,
                                         dx : dx + RH]
                            nc.tensor.matmul(
                                g_ps[:],
                                _r(wg_sb[ct][:, tap * 128:(tap + 1) * 128]),
                                _r(rhs), start=(tap == 0), stop=(tap == 8))
                        g_sb = tp.tile([128, NTW], BF, tag="gsb")
                        nc.scalar.activation(g_sb[:], g_ps[:], AF.Copy)
                        xg = tp.tile([128, NTW], BF, tag="xg")
                        xsl = x_sb[ct][:, b * NPIX + nt * NTW :
                                       b * NPIX + (nt + 1) * NTW]
                        nc.vector.tensor_tensor(xg[:], xsl, g_sb[:], ALU.mult)
                        nc.tensor.matmul(tp_ps[:],
                                         _r(w0t_sb[:, ct * 8:(ct + 1) * 8]),
                                         _r(xg[:]), start=(ct == 0),
                                         stop=False)
                        nc.tensor.matmul(tp_ps[:],
                                         _r(w0bt_sb[:, ct * 8:(ct + 1) * 8]),
                                         _r(xsl), start=False,
                                         stop=(ct == 2))
                    tp_sb = tp.tile([8, NTW], F32, tag="tpsb")
                    nc.scalar.activation(tp_sb[:], tp_ps[:], AF.Copy)
                    nc.sync.dma_start(
                        ar_in[AR_T1P1 : AR_T1P1 + 8 * 2 * NPIX].rearrange(
                            "(o p) -> o p", o=8)[
                            :, b * NPIX + nt * NTW : b * NPIX + (nt + 1) * NTW],
                        tp_sb[:])

        # ================= AllReduce #2 =================
        nc.gpsimd.collective_compute(
            "AllReduce", ALU.add, replica_groups=[list(range(NC_N))],
            ins=[ar_in[:]], outs=[ar_out[:]])

        # ================= PHASE 2: finish (replicated) =================
        with ExitStack() as p2:
            pool = p2.enter_context(tc.tile_pool(name="ph2", bufs=1))
            big2 = p2.enter_context(tc.tile_pool(name="ph2big", bufs=1))
            tp = p2.enter_context(tc.tile_pool(name="ph2t", bufs=2))
            ps = p2.enter_context(tc.tile_pool(name="ph2ps", bufs=2,
                                               space="PSUM"))

            stat_f = tp.tile([128, 16], F32, tag="statf")
            misc_f = pool.tile([8, 4], F32)
            nc.sync.dma_start(stat_f[:],
                              ar_out[AR_STAT : AR_STAT + 128 * 16].rearrange(
                                  "(p c) -> p c", p=128))
            nc.sync.dma_start(misc_f[:],
                              ar_out[AR_MISC : AR_MISC + 32].rearrange(
                                  "(o c) -> o c", o=8))
            eps_sb = pool.tile([1, 1], F32)
            nc.vector.memset(eps_sb[:], float(EPS))
            tot_ps = ps.tile([1, 16], F32, tag="smallps")
            nc.tensor.matmul(tot_ps[:], ones_col[:, :], stat_f[:], start=True,
                             stop=True)
            tot = pool.tile([1, 16], F32)
            nc.vector.tensor_copy(tot[:], tot_ps[:])
            # combine over ct: s[k,b] = sum_ct tot[ct*4 + 2b + k]
            acc = pool.tile([1, 4], F32)
            nc.vector.tensor_tensor(acc[:], tot[:, 0:4], tot[:, 4:8], ALU.add)
            nc.vector.tensor_tensor(acc[:], acc[:], tot[:, 8:12], ALU.add)
            nelem = float(CM * NPIX)
            mu = pool.tile([1, B], F32)
            rho = pool.tile([1, B], F32)
            var = tp.tile([1, B], F32, tag="var")
            musq = tp.tile([1, B], F32, tag="musq")
            for b in range(B):
                nc.vector.tensor_scalar_mul(mu[:, b : b + 1],
                                            acc[:, 2 * b : 2 * b + 1],
                                            1.0 / nelem)
                nc.vector.tensor_scalar_mul(var[:, b : b + 1],
                                            acc[:, 2 * b + 1 : 2 * b + 2],
                                            1.0 / nelem)
            nc.vector.tensor_tensor(musq[:], mu[:], mu[:], ALU.mult)
            nc.vector.tensor_tensor(var[:], var[:], musq[:], ALU.subtract)
            sd = tp.tile([1, B], F32, tag="sd")
            nc.scalar.activation(sd[:], var[:], AF.Sqrt, bias=eps_sb[:])
            nc.vector.reciprocal(rho[:], sd[:])

            def bcast(src_ap, parts):
                bps = ps.tile([128, 1], F32, tag="smallps")
                nc.tensor.matmul(bps[0:parts, :], ones_row[:, 0:parts],
                                 src_ap, start=True, stop=True)
                sb = tp.tile([128, 1], F32, tag="bcsb")
                nc.vector.tensor_copy(sb[0:parts, :], bps[0:parts, :])
                return sb

            bias0_sb = pool.tile([8, 1], F32)
            nc.sync.dma_start(bias0_sb[:], E["bias0_c"][:])

            # ---- z0 ----
            z0 = pool.tile([8, 2 * NPIX], F32)
            t1p1v = ar_out[AR_T1P1 : AR_T1P1 + 8 * 2 * NPIX].rearrange(
                "(o p) -> o p", o=8)
            t2vv = ar_out[AR_T2V : AR_T2V + 16 * 2 * NPIX].rearrange(
                "(o p) -> o p", o=16)
            for b in range(B):
                t1p1 = big2.tile([8, NPIX], F32, tag="t1p1")
                t2_sb = big2.tile([8, NPIX], F32, tag="t2sb")
                v_sb = big2.tile([8, NPIX], F32, tag="vsb")
                nc.sync.dma_start(t1p1[:],
                                  t1p1v[:, b * NPIX:(b + 1) * NPIX])
                nc.sync.dma_start(t2_sb[:],
                                  t2vv[0:8, b * NPIX:(b + 1) * NPIX])
                nc.sync.dma_start(v_sb[:],
                                  t2vv[8:16, b * NPIX:(b + 1) * NPIX])
                rho_b = bcast(rho[:, b : b + 1], 8)
                rmu = tp.tile([1, 1], F32, tag="rmu")
                nc.vector.tensor_tensor(rmu[:], rho[:, b : b + 1],
                                        mu[:, b : b + 1], ALU.mult)
                nc.vector.tensor_scalar_mul(rmu[:], rmu[:], -1.0)
                nrmu_b = bcast(rmu[:], 8)
                cst = tp.tile([8, 1], F32, tag="cst")
                nc.vector.tensor_scalar(cst[:], misc_f[:, 0:1],
                                        nrmu_b[0:8, :], None, ALU.mult)
                nc.vector.tensor_tensor(cst[:], cst[:], misc_f[:, 1:2],
                                        ALU.add)
                nc.vector.tensor_tensor(cst[:], cst[:], bias0_sb[:], ALU.add)
                sl = slice(b * NPIX, (b + 1) * NPIX)
                tt = big2.tile([8, NPIX], F32, tag="zt1")
                nc.vector.tensor_scalar(tt[:], t1p1[:], rho_b[0:8, :], None,
                                        ALU.mult)
                t2s = big2.tile([8, NPIX], F32, tag="zt2")
                nc.vector.tensor_scalar(t2s[:], t2_sb[:], nrmu_b[0:8, :],
                                        None, ALU.mult)
                nc.vector.tensor_tensor(tt[:], tt[:], t2s[:], ALU.add)
                nc.vector.tensor_tensor(tt[:], tt[:], v_sb[:], ALU.add)
                nc.scalar.activation(tt[:], tt[:], AF.Exp, bias=cst[:])
                nc.scalar.activation(z0[:, sl], tt[:], AF.Ln, bias=1.0)

            # ---- small-layer helpers ----
            gb1b_sb = pool.tile([8, 2], F32)
            nc.sync.dma_start(gb1b_sb[:], E["gbias1"][:])
            gb2b_sb = pool.tile([16, 2], F32)
            nc.sync.dma_start(gb2b_sb[:], E["gbias2"][:])
            w1t_sb = pool.tile([8, 16], F32)
            nc.sync.dma_start(w1t_sb[:], E["w1t"][:])
            w2t_sb = pool.tile([16, 1], F32)
            nc.sync.dma_start(w2t_sb[:], E["w2t"][:])
            b1_sb = pool.tile([16, 1], F32)
            nc.sync.dma_start(b1_sb[:], E["b1_c"][:])
            b2_sb = pool.tile([1, 1], F32)
            nc.sync.dma_start(b2_sb[:], E["b2_c"][:])

            def layer_stats(z, ch):
                st = tp.tile([128, 4], F32, tag="lst")
                scr = big2.tile([16, NPIX], F32, tag="lscr")
                for b in range(B):
                    nc.vector.tensor_reduce(
                        st[0:ch, 2 * b : 2 * b + 1],
                        z[:, b * NPIX:(b + 1) * NPIX], AX.X, ALU.add)
                    nc.scalar.activation(
                        scr[0:ch, :], z[:, b * NPIX:(b + 1) * NPIX],
                        AF.Square,
                        accum_out=st[0:ch, 2 * b + 1 : 2 * b + 2])
                lt_ps = ps.tile([1, 4], F32, tag="smallps")
                nc.tensor.matmul(lt_ps[:], ones_col[0:ch, :], st[0:ch, :],
                                 start=True, stop=True)
                t4 = tp.tile([1, 4], F32, tag="lsttot")
                nc.vector.tensor_copy(t4[:], lt_ps[:])
                n = float(ch * NPIX)
                m_ = tp.tile([1, B], F32, tag="lmu")
                r_ = tp.tile([1, B], F32, tag="lrho")
                v_ = tp.tile([1, B], F32, tag="lvar")
                q_ = tp.tile([1, B], F32, tag="lmsq")
                for b in range(B):
                    nc.vector.tensor_scalar_mul(
                        m_[:, b : b + 1], t4[:, 2 * b : 2 * b + 1], 1.0 / n)
                    nc.vector.tensor_scalar_mul(
                        v_[:, b : b + 1], t4[:, 2 * b + 1 : 2 * b + 2],
                        1.0 / n)
                nc.vector.tensor_tensor(q_[:], m_[:], m_[:], ALU.mult)
                nc.vector.tensor_tensor(v_[:], v_[:], q_[:], ALU.subtract)
                s_ = tp.tile([1, B], F32, tag="lsd")
                nc.scalar.activation(s_[:], v_[:], AF.Sqrt, bias=eps_sb[:])
                nc.vector.reciprocal(r_[:], s_[:])
                return m_, r_

            def spade_small(z, ch_in, h_pad_, layer, gbt, gbias_sb, n_gb):
                mu_l, rho_l = layer_stats(z, ch_in)
                gbw = pool.tile([HID, 9 * n_gb], BF, name=f"gbw{layer}",
                                tag=f"gbw{layer}")
                nc.sync.dma_start(
                    gbw[:].rearrange("k (t c) -> k t c", t=9),
                    gbt[:, :, :].rearrange("t k c -> k t c"))
                y = pool.tile([16, 2 * NPIX], F32, tag="ybuf")
                hv = [h_pad_[:, b * NPAD:(b + 1) * NPAD].rearrange(
                    "k (y x) -> k y x", y=PW, x=PW) for b in range(B)]
                for b in range(B):
                    rho_b = bcast(rho_l[:, b : b + 1], ch_in)
                    nmr = tp.tile([1, 1], F32, tag="nmr")
                    nc.vector.tensor_tensor(nmr[:], rho_l[:, b : b + 1],
                                            mu_l[:, b : b + 1], ALU.mult)
                    nc.vector.tensor_scalar_mul(nmr[:], nmr[:], -1.0)
                    nmr_b = bcast(nmr[:], ch_in)
                    ln = big2.tile([16, NPIX], F32, tag="lnb")
                    nc.vector.tensor_scalar(
                        ln[0:ch_in, :], z[:, b * NPIX:(b + 1) * NPIX],
                        rho_b[0:ch_in, :], nmr_b[0:ch_in, :],
                        ALU.mult, ALU.add)
                    for nt in range(NT):
                        ga_ps = ps.tile([16, NTW], F32, tag="gaps", bufs=1)
                        be_ps = ps.tile([16, NTW], F32, tag="beps", bufs=1)
                        for tap in range(9):
                            dy, dx = tap // 3, tap % 3
                            rhs = hv[b][:, dy + nt * 8 : dy + nt * 8 + 8,
                                        dx : dx + RH]
                            nc.tensor.matmul(
                                ga_ps[0:ch_in, :],
                                _r(gbw[:, tap * n_gb : tap * n_gb + ch_in]),
                                _r(rhs), start=(tap == 0), stop=(tap == 8))
                            nc.tensor.matmul(
                                be_ps[0:ch_in, :],
                                _r(gbw[:, tap * n_gb + ch_in :
                                       (tap + 1) * n_gb]),
                                _r(rhs), start=(tap == 0), stop=(tap == 8))
                        ga_sb = tp.tile([16, NTW], F32, tag="gasb")
                        be_sb = tp.tile([16, NTW], F32, tag="besb")
                        nc.scalar.activation(
                            ga_sb[0:ch_in, :], ga_ps[0:ch_in, :], AF.Identity,
                            bias=gbias_sb[0:ch_in, 0:1])
                        nc.scalar.activation(
                            be_sb[0:ch_in, :], be_ps[0:ch_in, :], AF.Identity,
                            bias=gbias_sb[0:ch_in, 1:2])
                        ysl = y[0:ch_in, b * NPIX + nt * NTW :
                                b * NPIX + (nt + 1) * NTW]
                        lsl = ln[0:ch_in, nt * NTW : (nt + 1) * NTW]
                        nc.vector.tensor_tensor(ysl, lsl, ga_sb[0:ch_in, :],
                                                ALU.mult)
                        nc.vector.tensor_tensor(
                            ysl, ysl, be_sb[0:ch_in, :], ALU.add)
                return y

            h_pad = pool.tile([HID, 2 * NPAD], BF, tag="hpad12")
            u1 = load_u(1, pool, "u1")
            hconv(1, h_pad, u1, ps)
            y1 = spade_small(z0, 8, h_pad, 1, E["gb1t"], gb1b_sb, 16)
            z1 = pool.tile([16, 2 * NPIX], F32)
            for b in range(B):
                for nt in range(NT):
                    zp = ps.tile([16, NTW], F32, tag="zps")
                    nc.tensor.matmul(
                        zp[:], w1t_sb[:],
                        y1[0:8, b * NPIX + nt * NTW :
                           b * NPIX + (nt + 1) * NTW],
                        start=True, stop=True)
                    zex = tp.tile([16, NTW], F32, tag="zex")
                    nc.scalar.activation(zex[:], zp[:], AF.Exp, bias=b1_sb[:])
                    nc.scalar.activation(
                        z1[:, b * NPIX + nt * NTW :
                           b * NPIX + (nt + 1) * NTW],
                        zex[:], AF.Ln, bias=1.0)

            u2 = load_u(2, pool, "u2")
            hconv(2, h_pad, u2, ps)
            y2 = spade_small(z1, 16, h_pad, 2, E["gb2t"], gb2b_sb, 32)
            for b in range(B):
                for nt in range(NT):
                    zp = ps.tile([1, NTW], F32, tag="zps")
                    nc.tensor.matmul(
                        zp[:], w2t_sb[:],
                        y2[:, b * NPIX + nt * NTW :
                           b * NPIX + (nt + 1) * NTW],
                        start=True, stop=True)
                    ot = tp.tile([1, NTW], F32, tag="otile")
                    nc.scalar.activation(ot[:], zp[:], AF.Exp, bias=b2_sb[:])
                    nc.scalar.activation(ot[:], ot[:], AF.Ln, bias=1.0)
                    nc.sync.dma_start(
                        E["out"][b, 0].rearrange("y x -> (y x)")[
                            nt * NTW : (nt + 1) * NTW],
                        ot[:])


def _prep_inputs(inputs):
    R = resize_matrix(HIMG, RH)
    x = np.asarray(inputs["x_main"], np.float32)
    f_sem = np.asarray(inputs["f_sem"], np.float32)
    segmap = np.asarray(inputs["segmap"], np.int32)
    idx = np.arange(HP) * HIMG // HP
    seg_p2 = np.ascontiguousarray(
        segmap[:, idx][:, :, idx].reshape(2, NP)).astype(np.int32)
    fsemt = np.ascontiguousarray(f_sem.reshape(2, CD, NP).transpose(0, 2, 1))

    ws = [np.asarray(inputs[f"ws{l}"], np.float32) for l in range(3)]
    bs = np.stack([np.asarray(inputs[f"bs{l}"], np.float32)
                   for l in range(3)]).reshape(3, HID, 1)
    wst = [np.ascontiguousarray(w.reshape(HID, CD, 9).transpose(2, 1, 0))
           for w in ws]
    wg0 = np.asarray(inputs["wg0"], np.float32)
    wb0 = np.asarray(inputs["wb0"], np.float32)
    w0 = np.asarray(inputs["w0"], np.float32).reshape(8, CM)

    def pack_gb(wg, wb, nf):
        wgt = np.asarray(wg, np.float32).reshape(nf, HID, 9).transpose(2, 1, 0)
        wbt = np.asarray(wb, np.float32).reshape(nf, HID, 9).transpose(2, 1, 0)
        return np.ascontiguousarray(np.concatenate([wgt, wbt], axis=2))

    gb1t = pack_gb(inputs["wg1"], inputs["wb1"], 8)
    gb2t = pack_gb(inputs["wg2"], inputs["wb2"], 16)
    gbias1 = np.stack([1.0 + np.asarray(inputs["bg1"], np.float32),
                       np.asarray(inputs["bb1"], np.float32)], axis=1)
    gbias2 = np.stack([1.0 + np.asarray(inputs["bg2"], np.float32),
                       np.asarray(inputs["bb2"], np.float32)], axis=1)
    w1t = np.ascontiguousarray(
        np.asarray(inputs["w1"], np.float32).reshape(16, 8).T)
    w2t = np.ascontiguousarray(
        np.asarray(inputs["w2"], np.float32).reshape(1, 16).T)
    b1_c = np.asarray(inputs["bias1"], np.float32).reshape(16, 1)
    b2_c = np.asarray(inputs["bias2"], np.float32).reshape(1, 1)
    bias0_c = np.asarray(inputs["bias0"], np.float32).reshape(8, 1)

    maps = []
    for r in range(NC_N):
        c0 = r * CSH
        b_img = r // 4
        s0 = SSH * (r % 4)
        units = UNITS[r * UPC:(r + 1) * UPC] if r * UPC < 27 else []
        wst_units = np.zeros((UPC, CD, HID), np.float32)
        for i, (l, t) in enumerate(UNITS[r * UPC:min((r + 1) * UPC, 27)]):
            wst_units[i] = wst[l][t]
        m = {
            "seg_my": np.ascontiguousarray(segmap[b_img]),
            "seg_p2": seg_p2,
            "fsemt": fsemt,
            "r_yt": np.ascontiguousarray(R),
            "r_ytb": np.ascontiguousarray(R).astype(ml_dtypes.bfloat16),
            "sbase": np.ascontiguousarray(
                np.broadcast_to((s0 + np.arange(SSH, dtype=np.float32))[None,
                                :], (112, SSH))),
            "x_r": np.ascontiguousarray(
                x.reshape(2, CM, NPIX)[:, c0:c0 + CSH].transpose(1, 0, 2)
                .reshape(CSH, 2 * NPIX)).astype(ml_dtypes.bfloat16),
            "wg0t": np.ascontiguousarray(
                wg0[c0:c0 + CSH].reshape(CSH, HID, 9).transpose(2, 1, 0))
                .astype(ml_dtypes.bfloat16),
            "wg0_r": np.ascontiguousarray(
                wg0[c0:c0 + CSH].reshape(CSH, HID * 9)).astype(
                    ml_dtypes.bfloat16),
            "wb0_r": np.ascontiguousarray(
                wb0[c0:c0 + CSH].reshape(CSH, HID * 9)).astype(
                    ml_dtypes.bfloat16),
            "w0t_r": np.ascontiguousarray(w0[:, c0:c0 + CSH].T).astype(ml_dtypes.bfloat16),
            "bg0_r": np.asarray(inputs["bg0"],
                                np.float32)[c0:c0 + CSH].reshape(CSH, 1),
            "bb0_r": np.asarray(inputs["bb0"],
                                np.float32)[c0:c0 + CSH].reshape(CSH, 1),
            "wst_u": wst_units,
            "bs_all": np.ascontiguousarray(bs),
            "gb1t": gb1t.astype(ml_dtypes.bfloat16),
            "gb2t": gb2t.astype(ml_dtypes.bfloat16),
            "gbias1": np.ascontiguousarray(gbias1),
            "gbias2": np.ascontiguousarray(gbias2),
            "w1t": w1t, "w2t": w2t, "b1_c": b1_c, "b2_c": b2_c,
            "bias0_c": bias0_c,
        }
        maps.append(m)
    return maps


_NC_CACHE = {}


def kernel(**inputs):
    if "nc" not in _NC_CACHE:
        _NC_CACHE["nc"] = build_kernel()
    nc = _NC_CACHE["nc"]
    maps = _prep_inputs(inputs)
    res = run_bass_kernel_spmd(nc, maps, core_ids=list(range(NC_N)))
    return np.asarray(res.results[0]["out"])
